# revision 1
# baseline (speedup 1.0000x reference)
"""Causal self-attention (B=4, T=2048, C=1024, H=16, D=64) on 8 TRN2 cores.

Sharding: core = 2*b + hg  (b = batch 0..3, hg = head-group 0..1 of 8 heads).
Each core computes its batch's QKV projections for its 8 heads (tensor
parallel over wq/wk/wv rows), RMSNorm+RoPE, causal attention, and a partial
output projection over its head-group's wproj columns. The two partials per
batch are summed on the host.

Device pipeline per core (all matmuls fp32r ~= tf32 @ 1 cyc/row):
  phase 1 (per 512-token chunk): PE-transpose x -> xT; q/k/v = xT.T @ w;
     RMS+RoPE on q,k in natural layout; PE-transpose q,k -> qT,kT (feature
     major); v blended with lamb*v1 into a [128, 8, 65] layout whose 65th
     column is 1.0 (computes softmax denominators during the PV matmul).
  phase 2 (per query chunk qc, per head): S^T[kt] = kT_h.T @ qT_h
     (keys on partitions, queries free); P = exp(S/8) via ACT with causal
     masking (partial-width exp + memset + triangle multiply on diagonal
     blocks); PV: [v | 1].T @ P accumulates y^T (65, 512) incl. denominator
     row; normalize via reciprocal + K=1 ones-matmul broadcast; project with
     wprojT and DMA the partial out.

The ISA has ONE semaphore-wait slot per instruction; Tile emits more.
_legalize_waits() splits extras onto same-engine NoOps post-scheduling.
"""

import math
import os

import numpy as np

import concourse.bass as bass
import concourse.mybir as mybir
import concourse.tile as tile
from concourse import bass_utils
from concourse.masks import make_identity

F32 = mybir.dt.float32
F32R = mybir.dt.float32r
F16 = mybir.dt.float16

B, T, C, H, D = 4, 2048, 1024, 16, 64
HG = C // 2          # 512 features per head group (8 heads x 64)
NT = T // 128        # 16 t-tiles
NQ = T // 512        # 4 query/t chunks
EPS = 1.1920928955078125e-07
SCALE = 1.0 / math.sqrt(D)  # 0.125

_wsplit_counter = [0]


def _legalize_waits(nc):
    """Split multi-wait instructions into single-wait NoOp chains."""
    n = 0
    for f in nc.m.functions:
        for bb in f.blocks:
            new_list = []
            changed = False
            for inst in bb.instructions:
                si = inst.sync_info
                if si is not None and si.on_wait and len(si.on_wait) > 1:
                    waits = list(si.on_wait)
                    for w in waits[:-1]:
                        _wsplit_counter[0] += 1
                        new_list.append(mybir.InstNoOp(
                            name=f"WSPLIT-{_wsplit_counter[0]}",
                            engine=inst.engine, ins=[], outs=[],
                            sync_info=mybir.SyncInfo(on_wait=[w], on_update=[]),
                        ))
                    si.on_wait = waits[-1:]
                    changed = True
                    n += 1
                new_list.append(inst)
            if changed:
                bb.instructions = new_list
    return n


def _build(lam: float, phases=(1, 2)) -> bass.Bass:
    nc = bass.Bass("TRN2", target_bir_lowering=False, debug=False, num_devices=8)

    xb_d = nc.dram_tensor("xbT", [C, T], F16, kind="ExternalInput").ap()
    v1_d = nc.dram_tensor("v1b", [T, HG], F32, kind="ExternalInput").ap()
    wq_d = nc.dram_tensor("wqT", [C, HG], F16, kind="ExternalInput").ap()
    wk_d = nc.dram_tensor("wkT", [C, HG], F16, kind="ExternalInput").ap()
    wv_d = nc.dram_tensor("wvT", [C, HG], F16, kind="ExternalInput").ap()
    wp_d = nc.dram_tensor("wpT", [HG, C], F16, kind="ExternalInput").ap()
    cos_d = nc.dram_tensor("cosn", [T, 32], F16, kind="ExternalInput").ap()
    sin_d = nc.dram_tensor("sinn", [T, 32], F16, kind="ExternalInput").ap()
    tri_d = nc.dram_tensor("tri01", [128, 128], F32, kind="ExternalInput").ap()
    out_d = nc.dram_tensor("out", [T, C], F32, kind="ExternalOutput").ap()

    with tile.TileContext(nc) as tc:
        with (
            tc.tile_pool(name="const", bufs=1) as const,
            tc.tile_pool(name="pers", bufs=1) as pers,
        ):
            ident = const.tile([128, 128], F32)
            make_identity(nc, ident)
            identr = const.tile([128, 128], F32R)
            nc.scalar.copy(out=identr, in_=ident)
            tri01 = const.tile([128, 128], F32R)
            nc.gpsimd.dma_start(out=tri01, in_=tri_d)
            ones81 = const.tile([128, 8, 1], F32)
            nc.vector.memset(ones81, 1.0)
            ones_f = const.tile([1, 64], F32)
            nc.vector.memset(ones_f, 1.0)
            ones1x64 = const.tile([1, 64], F32R)
            nc.scalar.copy(out=ones1x64, in_=ones_f)
            epsc = const.tile([128, 1], F32)
            nc.vector.memset(epsc, EPS)

            # persistent transposed q/k (feature-major) and v tiles
            qT = [pers.tile([128, T], F32R, name=f"qT{j}", tag=f"qT{j}")
                  for j in range(4)]
            kT = [pers.tile([128, T], F32R, name=f"kT{j}", tag=f"kT{j}")
                  for j in range(4)]
            vsb = [pers.tile([128, 8, 65], F32R, name=f"v{t}", tag=f"v{t}")
                   for t in range(NT)]

            # ------- merged phase 1 + phase 2, chunk-interleaved ---------
            with (
                tc.tile_pool(name="p1", bufs=1) as p1,
                tc.tile_pool(name="p1ps", bufs=1, space="PSUM") as p1ps,
                tc.tile_pool(name="p2", bufs=1) as p2,
            ):
                wq_sb = p1.tile([128, 8, HG], F16)
                nc.sync.dma_start(
                    out=wq_sb, in_=wq_d.rearrange("(c p) i -> p c i", p=128))
                wk_sb = p1.tile([128, 8, HG], F16)
                nc.sync.dma_start(
                    out=wk_sb, in_=wk_d.rearrange("(c p) i -> p c i", p=128))
                wv_sb = p1.tile([128, 8, HG], F16)
                nc.sync.dma_start(
                    out=wv_sb, in_=wv_d.rearrange("(c p) i -> p c i", p=128))
                cos_sb = p1.tile([128, NT, 32], F16)
                nc.sync.dma_start(
                    out=cos_sb, in_=cos_d.rearrange("(n p) i -> p n i", p=128))
                sin_sb = p1.tile([128, NT, 32], F16)
                nc.sync.dma_start(
                    out=sin_sb, in_=sin_d.rearrange("(n p) i -> p n i", p=128))

                w_sb = {"q": wq_sb, "k": wk_sb, "v": wv_sb}

                def do_p1(tc4):
                    xT = p1.tile([128, 8, 512], F16, name="xT", tag="xT",
                                 bufs=2)
                    t0 = tc4 * 512
                    nc.sync.dma_start(
                        out=xT,
                        in_=xb_d[:, t0:t0 + 512].rearrange(
                            "(c p) t -> p c t", p=128))
                    for ts in range(4):
                        tg = tc4 * 4 + ts
                        for which in ("q", "k", "v"):
                            ps = p1ps.tile([128, 512], F32, name="qkvps",
                                           tag="qkvps", bufs=2)
                            for cc in range(8):
                                nc.tensor.matmul(
                                    ps,
                                    xT[:, cc, ts * 128:(ts + 1) * 128],
                                    w_sb[which][:, cc, :],
                                    start=(cc == 0), stop=(cc == 7))
                            if os.environ.get("P1MODE") == "mm":
                                continue
                            p3 = ps.rearrange("p (h d) -> p h d", h=8)
                            if which == "v":
                                v1t = p1.tile([128, HG], F32, name="v1t",
                                              tag="v1t", bufs=2)
                                nc.sync.dma_start(
                                    out=v1t,
                                    in_=v1_d[tg * 128:(tg + 1) * 128, :])
                                v1s = v1t
                                nc.vector.scalar_tensor_tensor(
                                    out=vsb[tg][:, :, 0:64],
                                    in0=ps.rearrange("p (h d) -> p h d", h=8),
                                    scalar=1.0 - lam,
                                    in1=v1s.rearrange("p (h d) -> p h d", h=8),
                                    op0=mybir.AluOpType.mult,
                                    op1=mybir.AluOpType.add)
                                nc.vector.tensor_copy(
                                    out=vsb[tg][:, :, 64:65], in_=ones81)
                                continue

                            # ---- RoPE (psum -> rot sbuf) ----
                            rot = p1.tile([128, 512], F32R, name="rot",
                                          tag="rot", bufs=4)
                            r3 = rot.rearrange("p (h d) -> p h d", h=8)
                            tm = p1.tile([128, 256], F32, name="tm", tag="tm",
                                         bufs=2)
                            tm3 = tm.rearrange("p (h i) -> p h i", h=8)
                            c3 = cos_sb[:, tg, :].rearrange(
                                "p (o i) -> p o i", o=1).to_broadcast(
                                (128, 8, 32))
                            s3 = sin_sb[:, tg, :].rearrange(
                                "p (o i) -> p o i", o=1).to_broadcast(
                                (128, 8, 32))
                            x1 = p3[:, :, 0:32]
                            x2 = p3[:, :, 32:64]
                            nc.vector.tensor_mul(out=r3[:, :, 0:32], in0=x1,
                                                 in1=c3)
                            nc.vector.tensor_mul(out=tm3, in0=x2, in1=s3)
                            nc.gpsimd.tensor_add(out=r3[:, :, 0:32],
                                                 in0=r3[:, :, 0:32], in1=tm3)
                            nc.vector.tensor_mul(out=r3[:, :, 32:64], in0=x2,
                                                 in1=c3)
                            nc.vector.tensor_mul(out=tm3, in0=x1, in1=s3)
                            nc.gpsimd.tensor_sub(out=r3[:, :, 32:64],
                                                 in0=r3[:, :, 32:64], in1=tm3)
                            # ---- RMS stats (ACT square from psum,
                            # parallel with the rope ops) ----
                            sq = p1.tile([128, 512], F32, name="sq", tag="sq",
                                         bufs=2)
                            nc.scalar.square(out=sq, in_=ps)
                            ssum = p1.tile([128, 8], F32, name="ssum",
                                           tag="ssum", bufs=4)
                            nc.vector.tensor_reduce(
                                ssum, sq.rearrange("p (h d) -> p h d", h=8),
                                axis=mybir.AxisListType.X,
                                op=mybir.AluOpType.add)
                            srt = p1.tile([128, 8], F32, name="srt", tag="srt",
                                          bufs=4)
                            nc.scalar.activation(
                                srt, ssum, mybir.ActivationFunctionType.Sqrt,
                                bias=epsc, scale=1.0 / 64.0)
                            rst = p1.tile([128, 8], F32, name="rst", tag="rst",
                                          bufs=4)
                            nc.vector.reciprocal(out=rst, in_=srt)
                            rstb = rst.rearrange(
                                "p (h o) -> p h o", o=1).to_broadcast(
                                (128, 8, 64))
                            nc.gpsimd.tensor_mul(out=r3, in0=r3, in1=rstb)

                            if os.environ.get("P1MODE") == "rope":
                                continue
                            dstT = qT if which == "q" else kT
                            for j in range(4):
                                tp = p1ps.tile([128, 128], F32R, name="tpr",
                                               tag="tp", bufs=1)
                                nc.tensor.transpose(
                                    tp, rot[:, j * 128:(j + 1) * 128], identr)
                                nc.scalar.copy(
                                    out=dstT[j][:, tg * 128:(tg + 1) * 128],
                                    in_=tp)

                wp_sb = p2.tile([128, 4, C], F16)
                nc.sync.dma_start(
                    out=wp_sb, in_=wp_d.rearrange("(c p) j -> p c j", p=128))

                def do_p2(qc):
                    yT = [p2.tile([128, 512], F16, name=f"yT{j}",
                                  tag=f"yT{j}", bufs=2) for j in range(4)]
                    for hp in range(4):
                        pair = (2 * hp, 2 * hp + 1)
                        kts = list(range(4 * qc + 4))
                        pv = {}
                        for h in pair:
                            pv[h] = p1ps.tile([65, 512], F32, name="pv",
                                              tag="pv", bufs=2)
                        pt_live = {}

                        def emit_pv(kt, idx):
                            m = kt - 4 * qc
                            a0 = 128 * m if m in (1, 2) else 0
                            for h in pair:
                                nc.tensor.matmul(
                                    pv[h][:, a0:512], vsb[kt][:, h, :],
                                    pt_live.pop((kt, h))[:, a0:512],
                                    start=(idx == 0),
                                    stop=(idx == len(kts) - 1))

                        for idx, kt in enumerate(kts):
                            for h in pair:
                                b0 = 64 * (h % 2)
                                m = kt - 4 * qc
                                # matmul/exp window start (fp32r needs N>=256
                                # for full rate; m=3 gains nothing from N=128)
                                a0 = 0 if m < 1 else (128 * m if m < 3 else 0)
                                st = p1ps.tile([128, 512], F32, name="st",
                                               tag="st", bufs=3)
                                nc.tensor.matmul(
                                    st[:, a0:512],
                                    kT[hp][b0:b0 + 64,
                                           kt * 128:(kt + 1) * 128],
                                    qT[hp][b0:b0 + 64,
                                           qc * 512 + a0:(qc + 1) * 512],
                                    start=True, stop=True)
                                pt = p2.tile([128, 512], F32R, name="pt",
                                             tag="pt", bufs=8)
                                if m < 0:
                                    nc.scalar.activation(
                                        pt, st,
                                        mybir.ActivationFunctionType.Exp,
                                        scale=SCALE)
                                else:
                                    w0 = 128 * m
                                    nc.scalar.activation(
                                        pt[:, w0:512], st[:, w0:512],
                                        mybir.ActivationFunctionType.Exp,
                                        scale=SCALE)
                                    if m == 3:
                                        nc.gpsimd.memset(
                                            pt[:, 0:w0].bitcast(
                                                mybir.dt.uint32), 0)
                                    nc.vector.tensor_mul(
                                        out=pt[:, w0:w0 + 128],
                                        in0=pt[:, w0:w0 + 128], in1=tri01)
                                pt_live[(kt, h)] = pt
                            if idx >= 2:
                                emit_pv(kts[idx - 2], idx - 2)
                        if len(kts) >= 2:
                            emit_pv(kts[-2], len(kts) - 2)
                        emit_pv(kts[-1], len(kts) - 1)

                        for h in pair:
                            yu = p2.tile([65, 512], F32, name="yu", tag="yu",
                                         bufs=2)
                            nc.vector.tensor_copy(out=yu, in_=pv[h][0:65, :])
                            rec = p2.tile([1, 512], F32R, name="rec",
                                          tag="rec", bufs=2)
                            with nc.allow_low_precision(
                                    reason="softmax denom recip in f32r"):
                                nc.vector.reciprocal(out=rec,
                                                     in_=yu[64:65, :])
                            bc = p1ps.tile([64, 512], F32, name="bc",
                                           tag="qkvps", bufs=2)
                            nc.tensor.matmul(bc, ones1x64, rec, start=True,
                                             stop=True)
                            b0 = 64 * (h % 2)
                            nc.vector.tensor_mul(
                                out=yT[hp][b0:b0 + 64, :],
                                in0=yu[0:64, :], in1=bc)

                    for tsub in range(4):
                        for jc in range(2):
                            pr = p1ps.tile([128, 512], F32, name="pr",
                                           tag="qkvps", bufs=2)
                            for ft in range(4):
                                nc.tensor.matmul(
                                    pr,
                                    yT[ft][:, tsub * 128:(tsub + 1) * 128],
                                    wp_sb[:, ft, jc * 512:(jc + 1) * 512],
                                    start=(ft == 0), stop=(ft == 3))
                            osb = p2.tile([128, 512], F32, name="osb",
                                          tag="osb", bufs=3)
                            nc.scalar.copy(out=osb, in_=pr)
                            r0 = qc * 512 + tsub * 128
                            nc.sync.dma_start(
                                out=out_d[r0:r0 + 128,
                                          jc * 512:(jc + 1) * 512],
                                in_=osb)

                for ii in range(NQ):
                    if 1 in phases:
                        do_p1(ii)
                    if 2 in phases and ii >= 1:
                        do_p2(ii - 1)
                if 2 in phases:
                    do_p2(NQ - 1)

    _legalize_waits(nc)
    return nc


def _host_tables():
    inv_freq = 1.0 / (10000.0 ** (np.arange(0, D, 2, dtype=np.float32) / D))
    t = np.arange(T, dtype=np.float32)
    freqs = np.outer(t, inv_freq).astype(np.float32)      # (T, 32)
    cos16 = np.cos(freqs).astype(np.float16)
    sin16 = np.sin(freqs).astype(np.float16)
    cosn = cos16                                           # (T, 32)
    sinn = sin16
    p = np.arange(128)[:, None]
    f = np.arange(128)[None, :]
    tri = (p <= f).astype(np.float32)                      # (128, 128)
    return cosn, sinn, tri


_CACHE = {}


def kernel(x, v1, wq, wk, wv, wproj, lamb):
    x = np.asarray(x, dtype=np.float32)
    v1 = np.asarray(v1, dtype=np.float32)
    wq = np.asarray(wq, dtype=np.float32)
    wk = np.asarray(wk, dtype=np.float32)
    wv = np.asarray(wv, dtype=np.float32)
    wproj = np.asarray(wproj, dtype=np.float32)
    lam = float(np.asarray(lamb))

    cosn, sinn, tri = _host_tables()

    key = lam
    if key not in _CACHE:
        _CACHE[key] = _build(lam)
    nc = _CACHE[key]

    in_maps = []
    for core in range(8):
        b, hg = core // 2, core % 2
        sl = slice(hg * HG, (hg + 1) * HG)
        in_maps.append({
            "xbT": np.ascontiguousarray(x[b].T.astype(np.float16)),
            "v1b": np.ascontiguousarray(lam * v1[b][:, sl]),
            "wqT": np.ascontiguousarray(wq[sl, :].T.astype(np.float16)),
            "wkT": np.ascontiguousarray(wk[sl, :].T.astype(np.float16)),
            "wvT": np.ascontiguousarray(wv[sl, :].T.astype(np.float16)),
            "wpT": np.ascontiguousarray(wproj[:, sl].T.astype(np.float16)),
            "cosn": cosn,
            "sinn": sinn,
            "tri01": tri,
        })

    res = bass_utils.run_bass_kernel_spmd(nc, in_maps, core_ids=list(range(8)))
    y = np.empty((B, T, C), dtype=np.float32)
    for b in range(B):
        y[b] = res.results[2 * b]["out"] + res.results[2 * b + 1]["out"]
    return (y, v1)



# revision 29
# speedup vs baseline: 1.1638x; 1.1638x over previous
"""Causal self-attention (B=4, T=2048, C=1024, H=16, D=64) on 8 TRN2 cores.

Sharding: core = 2*b + hg  (b = batch 0..3, hg = head-group 0..1 of 8 heads).
Each core computes its batch's QKV projections for its 8 heads, RMSNorm+RoPE,
causal attention, and a partial output projection over its head-group's wproj
rows; the two partials per batch are summed on the host.

v2 pipeline (all matmuls fp16 @ 1 cyc/row):
  phase 1 (per 128-token tile): q/k/v = xT.T @ w (8 accum matmuls);
    RMS stats via ACT square + DVE reduce; RoPE+RMS fused as
    t1 = ps*(cos*rst), u = ps*(sin*rst), rot = [t1_0+u_1 | t1_1-u_0]
    (DVE muls, Pool add/sub); q/k transposed to feature-major via the
    XBAR DMA-transpose (zero PE cost); v blended with lam*v1 into
    [128, 8, 65] tiles whose 65th column is 1.0 (computes softmax
    denominators during the PV matmul).
  phase 2 (per query chunk qc, per head-pair): S^T = kT.T @ qT per head
    into one [128, 2, 512] PSUM tile; ONE merged exp for both heads
    (ACT); triangle mask multiply on diagonal blocks (DVE, fp16 4x);
    PV accumulates [65, W] per head with 2-step lag; denominators
    reciprocal (DVE) -> partition_broadcast (Pool, proxy library) ->
    normalize into fp16 yT; output projection + fp16 partial out DMA.

The ISA has ONE semaphore-wait slot per instruction; Tile emits more.
_legalize_waits() splits extras onto same-engine NoOps post-scheduling.
"""

import math

import numpy as np

import concourse.bass as bass
import concourse.mybir as mybir
import concourse.tile as tile
from concourse import bass_utils
from concourse import library_config

F32 = mybir.dt.float32
F16 = mybir.dt.float16

B, T, C, H, D = 4, 2048, 1024, 16, 64
HG = C // 2          # 512 features per head group (8 heads x 64)
NT = T // 128        # 16 t-tiles
NQ = T // 512        # 4 query/t chunks
EPS = 1.1920928955078125e-07
SCALE = 1.0 / math.sqrt(D)  # 0.125

_wsplit_counter = [0]


def _legalize_waits(nc):
    """Split multi-wait instructions into single-wait NoOp chains.

    DmaTransposeAnt cannot encode any sem wait in codegen - move ALL of
    its waits onto NoOps.
    """
    n = 0
    for f in nc.m.functions:
        for bb in f.blocks:
            new_list = []
            changed = False
            for inst in bb.instructions:
                si = inst.sync_info
                is_dt = type(inst).__name__ == "InstDmaTransposeAnt"
                keep = 0 if is_dt else 1
                if si is not None and si.on_wait and len(si.on_wait) > keep:
                    waits = list(si.on_wait)
                    for w in (waits if is_dt else waits[:-1]):
                        _wsplit_counter[0] += 1
                        new_list.append(mybir.InstNoOp(
                            name=f"WSPLIT-{_wsplit_counter[0]}",
                            engine=inst.engine, ins=[], outs=[],
                            sync_info=mybir.SyncInfo(on_wait=[w], on_update=[]),
                        ))
                    si.on_wait = [] if is_dt else waits[-1:]
                    changed = True
                    n += 1
                new_list.append(inst)
            if changed:
                bb.instructions = new_list
    return n


def _build(lam: float) -> bass.Bass:
    nc = bass.Bass("TRN2", target_bir_lowering=False, debug=False,
                   num_devices=8)

    xb_d = nc.dram_tensor("xbT", [C, T], F16, kind="ExternalInput").ap()
    v1_d = nc.dram_tensor("v1h", [T, HG], F16, kind="ExternalInput").ap()
    wq_d = nc.dram_tensor("wqT", [C, HG], F16, kind="ExternalInput").ap()
    wk_d = nc.dram_tensor("wkT", [C, HG], F16, kind="ExternalInput").ap()
    wv_d = nc.dram_tensor("wvT", [C, HG], F16, kind="ExternalInput").ap()
    wp_d = nc.dram_tensor("wpT", [HG, C], F16, kind="ExternalInput").ap()
    cos_d = nc.dram_tensor("cosn", [T, 32], F16, kind="ExternalInput").ap()
    sin_d = nc.dram_tensor("sinn", [T, 32], F16, kind="ExternalInput").ap()
    tri_d = nc.dram_tensor("tri01", [128, 128], F16, kind="ExternalInput").ap()
    out_d = nc.dram_tensor("out", [T, C], F16, kind="ExternalOutput").ap()

    with tile.TileContext(nc) as tc:
        with (
            tc.tile_pool(name="const", bufs=1) as const,
            tc.tile_pool(name="pers", bufs=1) as pers,
        ):
            tri01 = const.tile([128, 128], F16)
            nc.gpsimd.dma_start(out=tri01, in_=tri_d)
            ones81 = const.tile([128, 8, 1], F16)
            nc.vector.memset(ones81, 1.0)
            epsc = const.tile([128, 1], F32)
            nc.vector.memset(epsc, EPS)
            # ones row: broadcast lhsT expanding a [1,512] reciprocal row
            # to 64 partitions via matmul
            ones64 = const.tile([1, 64], F16)
            nc.vector.memset(ones64, 1.0)

            # persistent feature-major q/k and v tiles
            qT = pers.tile([128, 4, T], F16, name="qT", tag="qT")
            kT = pers.tile([128, 4, T], F16, name="kT", tag="kT")
            vsb = [pers.tile([128, 8, 65], F16, name=f"v{t}", tag=f"v{t}")
                   for t in range(NT)]

            with (
                tc.tile_pool(name="p1", bufs=1) as p1,
                tc.tile_pool(name="p1ps", bufs=1, space="PSUM") as p1ps,
                tc.tile_pool(name="p2", bufs=1) as p2,
            ):
                # stagger input loads across DMA queues; defer wp (first
                # needed ~50us in, at the end of p2(0)).  wq is split in
                # half so the first matmul group can start early.
                wq_sb = p1.tile([128, 8, HG], F16)
                wq_r = wq_d.rearrange("(c p) i -> p c i", p=128)
                nc.scalar.dma_start(out=wq_sb[:, 0:1, :], in_=wq_r[:, 0:1, :])
                nc.scalar.dma_start(out=wq_sb[:, 1:4, :], in_=wq_r[:, 1:4, :])
                nc.scalar.dma_start(out=wq_sb[:, 4:8, :], in_=wq_r[:, 4:8, :])
                wk_sb = p1.tile([128, 8, HG], F16)
                nc.gpsimd.dma_start(
                    out=wk_sb, in_=wk_d.rearrange("(c p) i -> p c i", p=128))
                wv_sb = p1.tile([128, 8, HG], F16)
                nc.scalar.dma_start(
                    out=wv_sb, in_=wv_d.rearrange("(c p) i -> p c i", p=128))
                cos_sb = p1.tile([128, NT, 32], F16)
                nc.gpsimd.dma_start(
                    out=cos_sb, in_=cos_d.rearrange("(n p) i -> p n i", p=128))
                sin_sb = p1.tile([128, NT, 32], F16)
                nc.gpsimd.dma_start(
                    out=sin_sb, in_=sin_d.rearrange("(n p) i -> p n i", p=128))
                wp_sb = p2.tile([128, 4, C], F16)
                wp_loaded = [False]

                w_sb = {"q": wq_sb, "k": wk_sb, "v": wv_sb}

                def p1_gen(tc4, use_st=False):
                    xT = p1.tile([128, 8, 512], F16, name="xT", tag="xT",
                                 bufs=2)
                    t0 = tc4 * 512
                    xr = xb_d[:, t0:t0 + 512].rearrange(
                        "(c p) t -> p c t", p=128)
                    nc.sync.dma_start(out=xT[:, 0:4, :], in_=xr[:, 0:4, :])
                    nc.sync.dma_start(out=xT[:, 4:8, :], in_=xr[:, 4:8, :])
                    unit = 0
                    for which in ("q", "k", "v"):
                        for ts in range(4):
                            tg = tc4 * 4 + ts
                            unit += 1
                            yield
                            if use_st and unit % 2 == 0:
                                ps = p1ps.tile([128, 2, 512], F32, name="st",
                                               tag="st", bufs=2)[:, 0, :]
                            else:
                                ps = p1ps.tile([128, 512], F32, name="qkvps",
                                               tag="qkvps", bufs=2)
                            for cc in range(8):
                                nc.tensor.matmul(
                                    ps,
                                    xT[:, cc, ts * 128:(ts + 1) * 128],
                                    w_sb[which][:, cc, :],
                                    start=(cc == 0), stop=(cc == 7))
                            p3 = ps.rearrange("p (h d) -> p h d", h=8)
                            if which == "v":
                                v1t = p1.tile([128, HG], F16, name="v1t",
                                              tag="v1t", bufs=2)
                                nc.sync.dma_start(
                                    out=v1t,
                                    in_=v1_d[tg * 128:(tg + 1) * 128, :])
                                nc.vector.scalar_tensor_tensor(
                                    out=vsb[tg][:, :, 0:64],
                                    in0=p3,
                                    scalar=1.0 - lam,
                                    in1=v1t.rearrange("p (h d) -> p h d", h=8),
                                    op0=mybir.AluOpType.mult,
                                    op1=mybir.AluOpType.add)
                                nc.vector.tensor_copy(
                                    out=vsb[tg][:, :, 64:65], in_=ones81)
                                continue

                            # ps is freed by two quick DVE ops (t1, u); RMS
                            # stats come from t1^2+u^2 = ps^2*(c^2+s^2).
                            c3 = cos_sb[:, tg, :].rearrange(
                                "p (o i) -> p o i", o=1).rearrange(
                                "p h (o d) -> p h o d", o=1).to_broadcast(
                                (128, 8, 2, 32))
                            s3 = sin_sb[:, tg, :].rearrange(
                                "p (o i) -> p o i", o=1).rearrange(
                                "p h (o d) -> p h o d", o=1).to_broadcast(
                                (128, 8, 2, 32))
                            p4 = ps.rearrange("p (h two d) -> p h two d",
                                              two=2, d=32)
                            t1 = p1.tile([128, 8, 2, 32], F16, name="t1",
                                         tag="t1", bufs=2)
                            nc.vector.tensor_mul(out=t1, in0=p4, in1=c3)
                            u = p1.tile([128, 8, 2, 32], F16, name="u",
                                        tag="u", bufs=2)
                            nc.vector.tensor_mul(out=u, in0=p4, in1=s3)
                            # RMS stats: ssum = sum(t1^2 + u^2) per head
                            sq = p1.tile([128, 8, 2, 32], F16, name="sq",
                                         tag="sq", bufs=2)
                            nc.vector.tensor_mul(out=sq, in0=t1, in1=t1)
                            squ = p1.tile([128, 8, 2, 32], F16, name="squ",
                                          tag="squ", bufs=2)
                            nc.vector.tensor_mul(out=squ, in0=u, in1=u)
                            s2 = p1.tile([128, 8, 2, 32], F16, name="s2",
                                         tag="s2", bufs=2)
                            nc.vector.tensor_add(out=s2, in0=sq, in1=squ)
                            ssum = p1.tile([128, 8], F32, name="ssum",
                                           tag="ssum", bufs=4)
                            nc.vector.tensor_reduce(
                                ssum, s2, axis=mybir.AxisListType.XY,
                                op=mybir.AluOpType.add)
                            srt = p1.tile([128, 8], F32, name="srt", tag="srt",
                                          bufs=4)
                            nc.scalar.activation(
                                srt, ssum, mybir.ActivationFunctionType.Sqrt,
                                bias=epsc, scale=1.0 / 64.0)
                            rst = p1.tile([128, 8], F32, name="rst", tag="rst",
                                          bufs=4)
                            nc.vector.reciprocal(out=rst, in_=srt)
                            # rot = [t1_0 + u_1 | t1_1 - u_0]  (Pool)
                            rot = p1.tile([128, 8, 2, 32], F16,
                                          name=f"rot{which}",
                                          tag=f"rot{which}", bufs=2)
                            nc.gpsimd.tensor_add(
                                out=rot[:, :, 0, :], in0=t1[:, :, 0, :],
                                in1=u[:, :, 1, :])
                            nc.gpsimd.tensor_sub(
                                out=rot[:, :, 1, :], in0=t1[:, :, 1, :],
                                in1=u[:, :, 0, :])
                            # apply 1/rms (Pool), then DMA-transpose
                            rstb = rst.rearrange(
                                "p (h o) -> p h o", o=1).rearrange(
                                "p h (o d) -> p h o d", o=1).to_broadcast(
                                (128, 8, 2, 32))
                            rot2 = p1.tile([128, 8, 2, 32], F16,
                                           name=f"rr{which}",
                                           tag=f"rr{which}", bufs=2)
                            nc.gpsimd.tensor_mul(out=rot2, in0=rot, in1=rstb)
                            dstT = qT if which == "q" else kT
                            nc.sync.dma_start_transpose(
                                out=dstT[:, :, tg * 128:(tg + 1) * 128],
                                in_=rot2.rearrange("p h two d -> p (h two d)"))

                def p2_gen(qc):
                    yT = p2.tile([128, 4, 512], F16, name="yT", tag="yT",
                                 bufs=2)
                    for hp in range(4):
                        pair = (2 * hp, 2 * hp + 1)
                        kts = list(range(4 * qc + 4))
                        pv = {}
                        for h in pair:
                            pv[h] = p1ps.tile([65, 512], F32, name="pv",
                                              tag="pv", bufs=2)
                        pt_live = {}

                        def emit_pv(kt, idx):
                            m = kt - 4 * qc
                            e0 = 128 * m if m > 0 else 0
                            pt = pt_live.pop(kt)
                            for si, h in enumerate(pair):
                                nc.tensor.matmul(
                                    pv[h][:, e0:512], vsb[kt][:, h, :],
                                    pt[:, si, e0:512],
                                    start=(idx == 0),
                                    stop=(idx == len(kts) - 1))

                        for idx, kt in enumerate(kts):
                            m = kt - 4 * qc
                            a0 = 128 * m if m > 0 else 0
                            st2 = p1ps.tile([128, 2, 512], F32, name="st",
                                            tag="st", bufs=2)
                            for si in range(2):
                                b0 = 64 * si
                                nc.tensor.matmul(
                                    st2[:, si, a0:512],
                                    kT[b0:b0 + 64, hp,
                                       kt * 128:(kt + 1) * 128],
                                    qT[b0:b0 + 64, hp,
                                       qc * 512 + a0:(qc + 1) * 512],
                                    start=True, stop=True)
                            pt = p2.tile([128, 2, 512], F16, name="pt",
                                         tag="pt", bufs=8)
                            nc.scalar.activation(
                                pt[:, :, a0:512], st2[:, :, a0:512],
                                mybir.ActivationFunctionType.Exp,
                                scale=SCALE)
                            if m >= 0:
                                w0 = 128 * m
                                for si in range(2):
                                    nc.vector.tensor_mul(
                                        out=pt[:, si, w0:w0 + 128],
                                        in0=pt[:, si, w0:w0 + 128],
                                        in1=tri01)
                            pt_live[kt] = pt
                            if idx >= 2:
                                emit_pv(kts[idx - 2], idx - 2)
                            yield
                        if len(kts) >= 2:
                            emit_pv(kts[-2], len(kts) - 2)
                        emit_pv(kts[-1], len(kts) - 1)

                        bc_ps = p1ps.tile([128, 512], F32, name="bcps",
                                          tag="qkvps", bufs=2)
                        for si, h in enumerate(pair):
                            b0 = 64 * si
                            rec = p2.tile([1, 512], F16, name="rec",
                                          tag="rec", bufs=4)
                            with nc.allow_low_precision(
                                    reason="softmax denom recip in fp16"):
                                nc.vector.reciprocal(out=rec,
                                                     in_=pv[h][64:65, :])
                            nc.tensor.matmul(bc_ps[b0:b0 + 64, :], ones64,
                                             rec, start=True, stop=True)
                        bc = p2.tile([128, 512], F16, name="bc", tag="bc",
                                     bufs=2)
                        nc.vector.tensor_copy(out=bc, in_=bc_ps)
                        for si, h in enumerate(pair):
                            b0 = 64 * si
                            nc.vector.tensor_mul(
                                out=yT[b0:b0 + 64, hp, :],
                                in0=pv[h][0:64, :], in1=bc[b0:b0 + 64, :])

                    for tsub in range(4):
                        for jc in range(2):
                            prt = p1ps.tile([128, 2, 512], F32, name="pr",
                                            tag="st", bufs=2)
                            pr = prt[:, 0, :]
                            for ft in range(4):
                                nc.tensor.matmul(
                                    pr,
                                    yT[:, ft, tsub * 128:(tsub + 1) * 128],
                                    wp_sb[:, ft, jc * 512:(jc + 1) * 512],
                                    start=(ft == 0), stop=(ft == 3))
                            osb = p2.tile([128, 512], F16, name="osb",
                                          tag="osb", bufs=3)
                            nc.scalar.copy(out=osb, in_=pr)
                            r0 = qc * 512 + tsub * 128
                            nc.sync.dma_start(
                                out=out_d[r0:r0 + 128,
                                          jc * 512:(jc + 1) * 512],
                                in_=osb)
                            yield

                # fine-grained software pipeline: run p1(0), then for each
                # qc interleave p1(qc+1) units into p2(qc)'s step stream so
                # PE always has QKV work to fill exp-limited bubbles.
                for _ in p1_gen(0, use_st=True):
                    pass
                for qc in range(NQ):
                    g2 = p2_gen(qc)
                    g1 = p1_gen(qc + 1) if qc + 1 < NQ else None
                    if g1 is not None:
                        next(g1, None)     # issue xT prefetch DMA now
                    steps = 4 * (4 * qc + 4) + 8
                    period = max(1, round(steps / 13.0))
                    i = 0
                    if not wp_loaded[0]:
                        nc.sync.dma_start(
                            out=wp_sb,
                            in_=wp_d.rearrange("(c p) j -> p c j", p=128))
                        wp_loaded[0] = True
                    for _ in g2:
                        i += 1
                        if g1 is not None and i % period == 0:
                            next(g1, None)
                    if g1 is not None:
                        for _ in g1:
                            pass

    _legalize_waits(nc)
    return nc


def _host_tables():
    inv_freq = 1.0 / (10000.0 ** (np.arange(0, D, 2, dtype=np.float32) / D))
    t = np.arange(T, dtype=np.float32)
    freqs = np.outer(t, inv_freq).astype(np.float32)      # (T, 32)
    cos16 = np.cos(freqs).astype(np.float16)
    sin16 = np.sin(freqs).astype(np.float16)
    p = np.arange(128)[:, None]
    f = np.arange(128)[None, :]
    tri = (p <= f).astype(np.float16)                      # (128, 128)
    return cos16, sin16, tri


_CACHE = {}


def kernel(x, v1, wq, wk, wv, wproj, lamb):
    x = np.asarray(x, dtype=np.float32)
    v1 = np.asarray(v1, dtype=np.float32)
    wq = np.asarray(wq, dtype=np.float32)
    wk = np.asarray(wk, dtype=np.float32)
    wv = np.asarray(wv, dtype=np.float32)
    wproj = np.asarray(wproj, dtype=np.float32)
    lam = float(np.asarray(lamb))

    cosn, sinn, tri = _host_tables()

    key = lam
    if key not in _CACHE:
        _CACHE[key] = _build(lam)
    nc = _CACHE[key]

    in_maps = []
    for core in range(8):
        b, hg = core // 2, core % 2
        sl = slice(hg * HG, (hg + 1) * HG)
        in_maps.append({
            "xbT": np.ascontiguousarray(x[b].T.astype(np.float16)),
            "v1h": np.ascontiguousarray(
                (lam * v1[b][:, sl]).astype(np.float16)),
            "wqT": np.ascontiguousarray(wq[sl, :].T.astype(np.float16)),
            "wkT": np.ascontiguousarray(wk[sl, :].T.astype(np.float16)),
            "wvT": np.ascontiguousarray(wv[sl, :].T.astype(np.float16)),
            "wpT": np.ascontiguousarray(wproj[:, sl].T.astype(np.float16)),
            "cosn": cosn,
            "sinn": sinn,
            "tri01": tri,
        })

    res = bass_utils.run_bass_kernel_spmd(nc, in_maps, core_ids=list(range(8)))
    y = np.empty((B, T, C), dtype=np.float32)
    for b in range(B):
        y[b] = (res.results[2 * b]["out"].astype(np.float32)
                + res.results[2 * b + 1]["out"].astype(np.float32))
    return (y, v1)


# revision 34
# speedup vs baseline: 1.2215x; 1.0496x over previous
"""Causal self-attention (B=4, T=2048, C=1024, H=16, D=64) on 8 TRN2 cores.

Sharding: core = 2*b + hg  (b = batch 0..3, hg = head-group 0..1 of 8 heads).
Each core computes its batch's QKV projections for its 8 heads, RMSNorm+RoPE,
causal attention, and a partial output projection over its head-group's wproj
rows; the two partials per batch are summed on the host.

v3 pipeline:
  QKV projections run as split-high/low fp8e4m3 DoubleRow matmuls
  (x = xh + xl, w = wh + wl quantized on host; ps = xh@wh + xh@wl + xl@wh,
  ~0.3% rms error, 0.75x the fp16 matmul cost).  RMS scale invariance
  absorbs the w*16 fp8 range scaling for q/k; the v-blend divides by 16.
  RoPE+RMS: t1 = ps*cos, u = ps*sin (DVE, frees the PSUM quickly), RMS
  stats from t1^2+u^2 = ps^2 (DVE fp16), rot = [t1_0+u_1 | t1_1-u_0]
  (Pool), 1/rms applied on Pool, then feature-major transpose via the
  XBAR DMA-transpose engine (zero PE cost).
  Attention per (query chunk qc, head pair hp): S^T per head into one
  [128, 2, 512] PSUM tile, ONE merged exp for both heads (ACT), fp16
  triangle mask on diagonal blocks (DVE 4x), PV accumulates [65, W] per
  head 3 steps behind; the 65th ones-column of v computes softmax
  denominators in the same matmul.  Each head-pair's normalization
  (reciprocal -> ones-matmul broadcast -> fp16 yT) is deferred into the
  next pair's loop; the output projection of chunk qc is interleaved
  into chunk qc+1's attention steps, as are the next chunk's QKV units,
  so PE always has fill work during exp-limited stretches.

The ISA has ONE semaphore-wait slot per instruction; Tile emits more.
_legalize_waits() splits extras onto same-engine NoOps post-scheduling.
DmaTransposeAnt cannot encode any wait - all its waits move to NoOps.
"""

import math

import numpy as np
import ml_dtypes

import concourse.bass as bass
import concourse.mybir as mybir
import concourse.tile as tile
from concourse import bass_utils

F32 = mybir.dt.float32
F16 = mybir.dt.float16
F8 = mybir.dt.float8e4

B, T, C, H, D = 4, 2048, 1024, 16, 64
HG = C // 2          # 512 features per head group (8 heads x 64)
NT = T // 128        # 16 t-tiles
NQ = T // 512        # 4 query/t chunks
EPS = 1.1920928955078125e-07
SCALE = 1.0 / math.sqrt(D)  # 0.125

_wsplit_counter = [0]


def _legalize_waits(nc):
    """Split multi-wait instructions into single-wait NoOp chains."""
    n = 0
    for f in nc.m.functions:
        for bb in f.blocks:
            new_list = []
            changed = False
            for inst in bb.instructions:
                si = inst.sync_info
                is_dt = type(inst).__name__ == "InstDmaTransposeAnt"
                keep = 0 if is_dt else 1
                if si is not None and si.on_wait and len(si.on_wait) > keep:
                    waits = list(si.on_wait)
                    for w in (waits if is_dt else waits[:-1]):
                        _wsplit_counter[0] += 1
                        new_list.append(mybir.InstNoOp(
                            name=f"WSPLIT-{_wsplit_counter[0]}",
                            engine=inst.engine, ins=[], outs=[],
                            sync_info=mybir.SyncInfo(on_wait=[w], on_update=[]),
                        ))
                    si.on_wait = [] if is_dt else waits[-1:]
                    changed = True
                    n += 1
                new_list.append(inst)
            if changed:
                bb.instructions = new_list
    return n


def _build(lam: float) -> bass.Bass:
    nc = bass.Bass("TRN2", target_bir_lowering=False, debug=False,
                   num_devices=8)

    xh_d = nc.dram_tensor("xTh", [C, T], F8, kind="ExternalInput").ap()
    xl_d = nc.dram_tensor("xTl", [C, T], F8, kind="ExternalInput").ap()
    v1_d = nc.dram_tensor("v1h", [T, HG], F16, kind="ExternalInput").ap()
    w_d = {}
    for wn in ("q", "k", "v"):
        for piece in ("h", "l"):
            w_d[wn + piece] = nc.dram_tensor(
                f"w{wn}{piece}", [C, HG], F8, kind="ExternalInput").ap()
    wp_d = nc.dram_tensor("wpT", [HG, C], F16, kind="ExternalInput").ap()
    cos_d = nc.dram_tensor("cosn", [T, 32], F16, kind="ExternalInput").ap()
    sin_d = nc.dram_tensor("sinn", [T, 32], F16, kind="ExternalInput").ap()
    tri_d = nc.dram_tensor("tri01", [128, 128], F16, kind="ExternalInput").ap()
    out_d = nc.dram_tensor("out", [T, C], F16, kind="ExternalOutput").ap()

    with tile.TileContext(nc) as tc:
        with (
            tc.tile_pool(name="const", bufs=1) as const,
            tc.tile_pool(name="pers", bufs=1) as pers,
        ):
            tri01 = const.tile([128, 128], F16)
            nc.gpsimd.dma_start(out=tri01, in_=tri_d)
            ones81 = const.tile([128, 8, 1], F16)
            nc.vector.memset(ones81, 1.0)
            epsc = const.tile([128, 1], F32)
            nc.vector.memset(epsc, EPS)
            ones64 = const.tile([1, 64], F16)
            nc.vector.memset(ones64, 1.0)

            # persistent feature-major q/k and v tiles
            qT = pers.tile([128, 4, T], F16, name="qT", tag="qT")
            kT = pers.tile([128, 4, T], F16, name="kT", tag="kT")
            vsb = [pers.tile([128, 8, 65], F16, name=f"v{t}", tag=f"v{t}")
                   for t in range(NT)]

            with (
                tc.tile_pool(name="p1", bufs=1) as p1,
                tc.tile_pool(name="p1ps", bufs=1, space="PSUM") as p1ps,
                tc.tile_pool(name="p2", bufs=1) as p2,
            ):
                # fp8 hi/lo weight tiles (0.5 MB each), staggered queues
                w_sb = {}
                for i, wn in enumerate(("qh", "ql", "kh", "kl", "vh", "vl")):
                    w_sb[wn] = p1.tile([128, 8, HG], F8, name=f"w{wn}")
                    eng = (nc.scalar, nc.gpsimd)[i % 2]
                    eng.dma_start(
                        out=w_sb[wn],
                        in_=w_d[wn[0] + wn[1]].rearrange(
                            "(c p) i -> p c i", p=128))
                cos_sb = p1.tile([128, NT, 32], F16)
                nc.gpsimd.dma_start(
                    out=cos_sb, in_=cos_d.rearrange("(n p) i -> p n i", p=128))
                sin_sb = p1.tile([128, NT, 32], F16)
                nc.scalar.dma_start(
                    out=sin_sb, in_=sin_d.rearrange("(n p) i -> p n i", p=128))
                wp_sb = p2.tile([128, 4, C], F16)
                wp_loaded = [False]

                def p1_gen(tc4):
                    xh = p1.tile([128, 8, 512], F8, name="xh", tag="xh",
                                 bufs=2)
                    xl = p1.tile([128, 8, 512], F8, name="xl", tag="xl",
                                 bufs=2)
                    t0 = tc4 * 512
                    nc.sync.dma_start(
                        out=xh, in_=xh_d[:, t0:t0 + 512].rearrange(
                            "(c p) t -> p c t", p=128))
                    nc.sync.dma_start(
                        out=xl, in_=xl_d[:, t0:t0 + 512].rearrange(
                            "(c p) t -> p c t", p=128))
                    for which in ("q", "k", "v"):
                        wh = w_sb[which + "h"]
                        wl = w_sb[which + "l"]
                        for ts in range(4):
                            tg = tc4 * 4 + ts
                            yield
                            ps = p1ps.tile([128, 512], F32, name="qkvps",
                                           tag="qkvps", bufs=2)
                            terms = ((xh, wh), (xh, wl), (xl, wh))
                            for ti, (xt, wt) in enumerate(terms):
                                for c in range(4):
                                    nc.tensor.matmul(
                                        ps,
                                        xt[:, 2 * c:2 * c + 2,
                                           ts * 128:(ts + 1) * 128],
                                        wt[:, 2 * c:2 * c + 2, :],
                                        start=(ti == 0 and c == 0),
                                        stop=(ti == 2 and c == 3),
                                        perf_mode=mybir.MatmulPerfMode
                                        .DoubleRow)
                            p3 = ps.rearrange("p (h d) -> p h d", h=8)
                            if which == "v":
                                v1t = p1.tile([128, HG], F16, name="v1t",
                                              tag="v1t", bufs=2)
                                nc.sync.dma_start(
                                    out=v1t,
                                    in_=v1_d[tg * 128:(tg + 1) * 128, :])
                                nc.vector.scalar_tensor_tensor(
                                    out=vsb[tg][:, :, 0:64],
                                    in0=p3,
                                    scalar=(1.0 - lam) / 16.0,
                                    in1=v1t.rearrange("p (h d) -> p h d", h=8),
                                    op0=mybir.AluOpType.mult,
                                    op1=mybir.AluOpType.add)
                                nc.vector.tensor_copy(
                                    out=vsb[tg][:, :, 64:65], in_=ones81)
                                continue

                            # ps freed by two quick DVE ops (t1, u); RMS
                            # stats from t1^2+u^2 = ps^2*(c^2+s^2) (scale
                            # invariant, so the 16x fp8 w-scale cancels).
                            c3 = cos_sb[:, tg, :].rearrange(
                                "p (o i) -> p o i", o=1).rearrange(
                                "p h (o d) -> p h o d", o=1).to_broadcast(
                                (128, 8, 2, 32))
                            s3 = sin_sb[:, tg, :].rearrange(
                                "p (o i) -> p o i", o=1).rearrange(
                                "p h (o d) -> p h o d", o=1).to_broadcast(
                                (128, 8, 2, 32))
                            p4 = ps.rearrange("p (h two d) -> p h two d",
                                              two=2, d=32)
                            t1 = p1.tile([128, 8, 2, 32], F16, name="t1",
                                         tag="t1", bufs=2)
                            nc.vector.tensor_mul(out=t1, in0=p4, in1=c3)
                            u = p1.tile([128, 8, 2, 32], F16, name="u",
                                        tag="u", bufs=2)
                            nc.vector.tensor_mul(out=u, in0=p4, in1=s3)
                            sq = p1.tile([128, 8, 2, 32], F16, name="sq",
                                         tag="sq", bufs=2)
                            nc.vector.tensor_mul(out=sq, in0=t1, in1=t1)
                            squ = p1.tile([128, 8, 2, 32], F16, name="squ",
                                          tag="squ", bufs=2)
                            nc.vector.tensor_mul(out=squ, in0=u, in1=u)
                            s2 = p1.tile([128, 8, 2, 32], F16, name="s2",
                                         tag="s2", bufs=2)
                            nc.vector.tensor_add(out=s2, in0=sq, in1=squ)
                            ssum = p1.tile([128, 8], F32, name="ssum",
                                           tag="ssum", bufs=4)
                            nc.vector.tensor_reduce(
                                ssum, s2, axis=mybir.AxisListType.XY,
                                op=mybir.AluOpType.add)
                            srt = p1.tile([128, 8], F32, name="srt", tag="srt",
                                          bufs=4)
                            nc.scalar.activation(
                                srt, ssum, mybir.ActivationFunctionType.Sqrt,
                                bias=epsc, scale=1.0 / 64.0)
                            rst = p1.tile([128, 8], F32, name="rst", tag="rst",
                                          bufs=4)
                            nc.vector.reciprocal(out=rst, in_=srt)
                            # rot = [t1_0 + u_1 | t1_1 - u_0]  (Pool)
                            rot = p1.tile([128, 8, 2, 32], F16,
                                          name=f"rot{which}",
                                          tag=f"rot{which}", bufs=2)
                            nc.gpsimd.tensor_add(
                                out=rot[:, :, 0, :], in0=t1[:, :, 0, :],
                                in1=u[:, :, 1, :])
                            nc.gpsimd.tensor_sub(
                                out=rot[:, :, 1, :], in0=t1[:, :, 1, :],
                                in1=u[:, :, 0, :])
                            # apply 1/rms (Pool), then DMA-transpose
                            rstb = rst.rearrange(
                                "p (h o) -> p h o", o=1).rearrange(
                                "p h (o d) -> p h o d", o=1).to_broadcast(
                                (128, 8, 2, 32))
                            rot2 = p1.tile([128, 8, 2, 32], F16,
                                           name=f"rr{which}",
                                           tag=f"rr{which}", bufs=2)
                            nc.gpsimd.tensor_mul(out=rot2, in0=rot, in1=rstb)
                            dstT = qT if which == "q" else kT
                            nc.sync.dma_start_transpose(
                                out=dstT[:, :, tg * 128:(tg + 1) * 128],
                                in_=rot2.rearrange("p h two d -> p (h two d)"))

                yT_of = {}
                pending = [None]

                def p2_gen(qc):
                    yT = p2.tile([128, 4, 512], F16, name="yT", tag="yT",
                                 bufs=2)
                    yT_of[qc] = yT
                    for hp in range(4):
                        pair = (2 * hp, 2 * hp + 1)
                        kts = list(range(4 * qc + 4))
                        lag = 3 if len(kts) > 3 else 2
                        pv = {}
                        for h in pair:
                            pv[h] = p1ps.tile([65, 512], F32, name="pv",
                                              tag="pv", bufs=2)
                        pt_live = {}

                        def emit_pv(kt, idx, pv=pv, pair=pair, kts=kts,
                                    qc=qc):
                            m = kt - 4 * qc
                            e0 = 128 * m if m > 0 else 0
                            pt = pt_live.pop(kt)
                            for si, h in enumerate(pair):
                                nc.tensor.matmul(
                                    pv[h][:, e0:512], vsb[kt][:, h, :],
                                    pt[:, si, e0:512],
                                    start=(idx == 0),
                                    stop=(idx == len(kts) - 1))

                        for idx, kt in enumerate(kts):
                            m = kt - 4 * qc
                            a0 = 128 * m if m > 0 else 0
                            st2 = p1ps.tile([128, 2, 512], F32, name="st",
                                            tag="st", bufs=2)
                            for si in range(2):
                                b0 = 64 * si
                                nc.tensor.matmul(
                                    st2[:, si, a0:512],
                                    kT[b0:b0 + 64, hp,
                                       kt * 128:(kt + 1) * 128],
                                    qT[b0:b0 + 64, hp,
                                       qc * 512 + a0:(qc + 1) * 512],
                                    start=True, stop=True)
                            pt = p2.tile([128, 2, 512], F16, name="pt",
                                         tag="pt", bufs=8)
                            nc.scalar.activation(
                                pt[:, :, a0:512], st2[:, :, a0:512],
                                mybir.ActivationFunctionType.Exp,
                                scale=SCALE)
                            if m >= 0:
                                w0 = 128 * m
                                for si in range(2):
                                    nc.vector.tensor_mul(
                                        out=pt[:, si, w0:w0 + 128],
                                        in0=pt[:, si, w0:w0 + 128],
                                        in1=tri01)
                            pt_live[kt] = pt
                            if idx >= lag:
                                emit_pv(kts[idx - lag], idx - lag)
                            if idx == 1 and pending[0] is not None:
                                pending[0]()
                                pending[0] = None
                            yield
                        for j in range(lag, 0, -1):
                            emit_pv(kts[-j], len(kts) - j)
                        if pending[0] is not None:
                            pending[0]()
                            pending[0] = None

                        def normalize(pv=pv, pair=pair, hp=hp, yT=yT):
                            bc_ps = p1ps.tile([128, 512], F32, name="bcps",
                                              tag="qkvps", bufs=2)
                            for si, h in enumerate(pair):
                                b0 = 64 * si
                                rec = p2.tile([1, 512], F16, name="rec",
                                              tag="rec", bufs=4)
                                with nc.allow_low_precision(
                                        reason="softmax denom recip fp16"):
                                    nc.vector.reciprocal(
                                        out=rec, in_=pv[h][64:65, :])
                                nc.tensor.matmul(bc_ps[b0:b0 + 64, :],
                                                 ones64, rec,
                                                 start=True, stop=True)
                            bc = p2.tile([128, 512], F16, name="bc", tag="bc",
                                         bufs=2)
                            nc.vector.tensor_copy(out=bc, in_=bc_ps)
                            for si, h in enumerate(pair):
                                b0 = 64 * si
                                nc.vector.tensor_mul(
                                    out=yT[b0:b0 + 64, hp, :],
                                    in0=pv[h][0:64, :], in1=bc[b0:b0 + 64, :])

                        pending[0] = normalize

                def proj_gen(qc):
                    yT = yT_of[qc]
                    for tsub in range(4):
                        for jc in range(2):
                            yield
                            pr = p1ps.tile([128, 512], F32, name="pr",
                                           tag="qkvps", bufs=2)
                            for ft in range(4):
                                nc.tensor.matmul(
                                    pr,
                                    yT[:, ft, tsub * 128:(tsub + 1) * 128],
                                    wp_sb[:, ft, jc * 512:(jc + 1) * 512],
                                    start=(ft == 0), stop=(ft == 3))
                            osb = p2.tile([128, 512], F16, name="osb",
                                          tag="osb", bufs=3)
                            nc.scalar.copy(out=osb, in_=pr)
                            r0 = qc * 512 + tsub * 128
                            nc.sync.dma_start(
                                out=out_d[r0:r0 + 128,
                                          jc * 512:(jc + 1) * 512],
                                in_=osb)

                # software pipeline: start q,k of chunk 0; interleave the
                # v units + next chunk's QKV + previous chunk's projection
                # into each attention chunk's kt-step stream.
                g0 = p1_gen(0)
                for _ in range(9):      # xh/xl DMA + all q and k units
                    next(g0)
                fill_counts = {0: 4 + 13, 1: 13 + 8, 2: 13 + 8, 3: 8}
                for qc in range(NQ):
                    chain = []
                    if qc == 0:
                        chain.append(g0)
                    if qc + 1 < NQ:
                        chain.append(p1_gen(qc + 1))
                    if qc >= 1:
                        chain.append(proj_gen(qc - 1))
                    if not wp_loaded[0]:
                        nc.sync.dma_start(
                            out=wp_sb,
                            in_=wp_d.rearrange("(c p) j -> p c j", p=128))
                        wp_loaded[0] = True
                    steps = 4 * (4 * qc + 4)
                    nfill = fill_counts[qc]
                    acc = [0.0]
                    rate = nfill / steps

                    def fire():
                        while chain:
                            try:
                                next(chain[0])
                                return
                            except StopIteration:
                                chain.pop(0)

                    i = 0
                    for _ in p2_gen(qc):
                        i += 1
                        acc[0] += rate
                        while acc[0] >= 1.0:
                            acc[0] -= 1.0
                            fire()
                    while chain:
                        fire()
                        if not chain:
                            break
                if pending[0] is not None:
                    pending[0]()
                    pending[0] = None
                for _ in proj_gen(NQ - 1):
                    pass

    _legalize_waits(nc)
    return nc


def _host_tables():
    inv_freq = 1.0 / (10000.0 ** (np.arange(0, D, 2, dtype=np.float32) / D))
    t = np.arange(T, dtype=np.float32)
    freqs = np.outer(t, inv_freq).astype(np.float32)      # (T, 32)
    cos16 = np.cos(freqs).astype(np.float16)
    sin16 = np.sin(freqs).astype(np.float16)
    p = np.arange(128)[:, None]
    f = np.arange(128)[None, :]
    tri = (p <= f).astype(np.float16)                      # (128, 128)
    return cos16, sin16, tri


def _hilo(a):
    hi = a.astype(ml_dtypes.float8_e4m3)
    lo = (a - hi.astype(np.float32)).astype(ml_dtypes.float8_e4m3)
    return hi, lo


_CACHE = {}


def kernel(x, v1, wq, wk, wv, wproj, lamb):
    x = np.asarray(x, dtype=np.float32)
    v1 = np.asarray(v1, dtype=np.float32)
    wq = np.asarray(wq, dtype=np.float32)
    wk = np.asarray(wk, dtype=np.float32)
    wv = np.asarray(wv, dtype=np.float32)
    wproj = np.asarray(wproj, dtype=np.float32)
    lam = float(np.asarray(lamb))

    cosn, sinn, tri = _host_tables()

    key = lam
    if key not in _CACHE:
        _CACHE[key] = _build(lam)
    nc = _CACHE[key]

    in_maps = []
    for core in range(8):
        b, hg = core // 2, core % 2
        sl = slice(hg * HG, (hg + 1) * HG)
        xh, xl = _hilo(np.ascontiguousarray(x[b].T))
        m = {
            "xTh": xh,
            "xTl": xl,
            "v1h": np.ascontiguousarray(
                (lam * v1[b][:, sl]).astype(np.float16)),
            "wpT": np.ascontiguousarray(wproj[:, sl].T.astype(np.float16)),
            "cosn": cosn,
            "sinn": sinn,
            "tri01": tri,
        }
        for wn, w in (("q", wq), ("k", wk), ("v", wv)):
            wh, wl = _hilo(np.ascontiguousarray(w[sl, :].T) * 16.0)
            m[f"w{wn}h"] = wh
            m[f"w{wn}l"] = wl
        in_maps.append(m)

    res = bass_utils.run_bass_kernel_spmd(nc, in_maps, core_ids=list(range(8)))
    y = np.empty((B, T, C), dtype=np.float32)
    for b in range(B):
        y[b] = (res.results[2 * b]["out"].astype(np.float32)
                + res.results[2 * b + 1]["out"].astype(np.float32))
    return (y, v1)


# revision 72
# speedup vs baseline: 1.3403x; 1.0973x over previous
"""Causal self-attention (B=4, T=2048, C=1024, H=16, D=64) on 8 TRN2 cores.

Sharding: core = 2*b + hg  (b = batch 0..3, hg = head-group 0..1 of 8 heads).
Each core computes its batch's QKV projections for its 8 heads, RMSNorm+RoPE,
causal attention, and a partial output projection over its head-group's wproj
rows; the two partials per batch are summed on the host.

v3 pipeline:
  QKV projections run as split-high/low fp8e4m3 DoubleRow matmuls
  (x = xh + xl, w = wh + wl quantized on host; ps = xh@wh + xh@wl + xl@wh,
  ~0.3% rms error, 0.75x the fp16 matmul cost).  RMS scale invariance
  absorbs the w*16 fp8 range scaling for q/k; the v-blend divides by 16.
  RoPE+RMS: t1 = ps*cos, u = ps*sin (DVE, frees the PSUM quickly), RMS
  stats from t1^2+u^2 = ps^2 (DVE fp16), rot = [t1_0+u_1 | t1_1-u_0]
  (Pool), 1/rms applied on Pool, then feature-major transpose via the
  XBAR DMA-transpose engine (zero PE cost).
  Attention per (query chunk qc, head pair hp): S^T per head into one
  [128, 2, 512] PSUM tile, ONE merged exp for both heads (ACT), fp16
  triangle mask on diagonal blocks (DVE 4x), PV accumulates [65, W] per
  head up to 6 steps behind; the 65th ones-column of v computes softmax
  denominators in the same matmul.  Each head-pair's normalization
  (reciprocal -> ones-matmul broadcast -> fp16 yT) is deferred into the
  next pair's loop; output projections are deferred into later chunks'
  attention steps (proj 0 -> chunk 1, proj 1 and 2 -> chunk 3), as are
  the next chunk's QKV units, so PE always has fill work during
  exp-limited stretches.

The ISA has ONE semaphore-wait slot per instruction; Tile emits more.
_legalize_waits() splits extras onto same-engine NoOps post-scheduling.
DmaTransposeAnt cannot encode any wait - all its waits move to NoOps.
"""

import math

import numpy as np
import ml_dtypes

import concourse.bass as bass
import concourse.mybir as mybir
import concourse.tile as tile
from concourse import bass_utils

F32 = mybir.dt.float32
F16 = mybir.dt.float16
F8 = mybir.dt.float8e4

B, T, C, H, D = 4, 2048, 1024, 16, 64
HG = C // 2          # 512 features per head group (8 heads x 64)
NT = T // 128        # 16 t-tiles
NQ = T // 512        # 4 query/t chunks
EPS = 1.1920928955078125e-07
SCALE = 1.0 / math.sqrt(D)  # 0.125

_wsplit_counter = [0]


def _legalize_waits(nc):
    """Split multi-wait instructions into single-wait NoOp chains."""
    n = 0
    for f in nc.m.functions:
        for bb in f.blocks:
            new_list = []
            changed = False
            for inst in bb.instructions:
                si = inst.sync_info
                is_dt = type(inst).__name__ == "InstDmaTransposeAnt"
                keep = 0 if is_dt else 1
                if si is not None and si.on_wait and len(si.on_wait) > keep:
                    waits = list(si.on_wait)
                    for w in (waits if is_dt else waits[:-1]):
                        _wsplit_counter[0] += 1
                        new_list.append(mybir.InstNoOp(
                            name=f"WSPLIT-{_wsplit_counter[0]}",
                            engine=inst.engine, ins=[], outs=[],
                            sync_info=mybir.SyncInfo(on_wait=[w], on_update=[]),
                        ))
                    si.on_wait = [] if is_dt else waits[-1:]
                    changed = True
                    n += 1
                new_list.append(inst)
            if changed:
                bb.instructions = new_list
    return n


def _build(lam: float) -> bass.Bass:
    nc = bass.Bass("TRN2", target_bir_lowering=False, debug=False,
                   num_devices=8)

    xh_d = nc.dram_tensor("xTh", [C, T], F8, kind="ExternalInput").ap()
    xl_d = nc.dram_tensor("xTl", [C, T], F8, kind="ExternalInput").ap()
    v1_d = nc.dram_tensor("v1h", [T, HG], F16, kind="ExternalInput").ap()
    w_d = {}
    for wn in ("q", "k", "v"):
        for piece in ("h", "l"):
            w_d[wn + piece] = nc.dram_tensor(
                f"w{wn}{piece}", [C, HG], F8, kind="ExternalInput").ap()
    wp_d = nc.dram_tensor("wpT", [HG, C], F16, kind="ExternalInput").ap()
    cos_d = nc.dram_tensor("cosn", [T, 32], F16, kind="ExternalInput").ap()
    sin_d = nc.dram_tensor("sinn", [T, 32], F16, kind="ExternalInput").ap()
    tri_d = nc.dram_tensor("tri01", [128, 128], F16, kind="ExternalInput").ap()
    out_d = nc.dram_tensor("out", [T, C], F16, kind="ExternalOutput").ap()

    with tile.TileContext(nc) as tc:
        with (
            tc.tile_pool(name="const", bufs=1) as const,
            tc.tile_pool(name="pers", bufs=1) as pers,
        ):
            tri01 = const.tile([128, 128], F16)
            nc.gpsimd.dma_start(out=tri01, in_=tri_d)
            ones81 = const.tile([128, 8, 1], F16)
            nc.vector.memset(ones81, 1.0)
            epsc = const.tile([128, 1], F32)
            nc.vector.memset(epsc, EPS)
            ones64 = const.tile([1, 64], F16)
            nc.vector.memset(ones64, 1.0)

            # persistent feature-major q/k and v tiles
            qT = pers.tile([128, 4, T], F16, name="qT", tag="qT")
            kT = pers.tile([128, 4, T], F16, name="kT", tag="kT")
            vsb = [pers.tile([128, 8, 65], F16, name=f"v{t}", tag=f"v{t}")
                   for t in range(NT)]

            with (
                tc.tile_pool(name="p1", bufs=1) as p1,
                tc.tile_pool(name="p1ps", bufs=1, space="PSUM") as p1ps,
                tc.tile_pool(name="p2", bufs=1) as p2,
            ):
                # chunk-0 x DMAs first: the very first matmul needs them
                x0h = p1.tile([128, 8, 512], F8, name="xh", tag="xh", bufs=2)
                x0l = p1.tile([128, 8, 512], F8, name="xl", tag="xl", bufs=2)
                nc.sync.dma_start(
                    out=x0h,
                    in_=xh_d[:, 0:512].rearrange("(c p) t -> p c t", p=128))
                nc.sync.dma_start(
                    out=x0l,
                    in_=xl_d[:, 0:512].rearrange("(c p) t -> p c t", p=128))

                w_sb = {}
                for i, wn in enumerate(("qh", "ql", "kh", "kl", "vh", "vl")):
                    w_sb[wn] = p1.tile([128, 8, HG], F8, name=f"w{wn}")
                    eng = (nc.scalar, nc.gpsimd)[i % 2]
                    wr = w_d[wn[0] + wn[1]].rearrange("(c p) i -> p c i",
                                                      p=128)
                    if wn in ("qh", "ql"):
                        eng.dma_start(out=w_sb[wn][:, 0:2, :],
                                      in_=wr[:, 0:2, :])
                        eng.dma_start(out=w_sb[wn][:, 2:8, :],
                                      in_=wr[:, 2:8, :])
                    else:
                        eng.dma_start(out=w_sb[wn], in_=wr)
                cos_sb = p1.tile([128, NT, 32], F16)
                nc.gpsimd.dma_start(
                    out=cos_sb, in_=cos_d.rearrange("(n p) i -> p n i", p=128))
                sin_sb = p1.tile([128, NT, 32], F16)
                nc.scalar.dma_start(
                    out=sin_sb, in_=sin_d.rearrange("(n p) i -> p n i", p=128))
                wp_sb = p2.tile([128, 4, C], F16)
                wp_loaded = [False]

                def p1_gen(tc4, xtiles=None, use_st=False):
                    if xtiles is not None:
                        xh, xl = xtiles
                    else:
                        xh = p1.tile([128, 8, 512], F8, name="xh", tag="xh",
                                     bufs=2)
                        xl = p1.tile([128, 8, 512], F8, name="xl", tag="xl",
                                     bufs=2)
                        t0 = tc4 * 512
                        nc.sync.dma_start(
                            out=xh, in_=xh_d[:, t0:t0 + 512].rearrange(
                                "(c p) t -> p c t", p=128))
                        nc.sync.dma_start(
                            out=xl, in_=xl_d[:, t0:t0 + 512].rearrange(
                                "(c p) t -> p c t", p=128))
                    for which in ("q", "k", "v"):
                        for ts in range(4):
                            tg = tc4 * 4 + ts
                            yield
                            wh = w_sb[which + "h"]
                            wl = w_sb[which + "l"]
                            if use_st and (ts % 2 == 0):
                                ps = p1ps.tile([128, 2, 512], F32, name="st",
                                               tag="st", bufs=2)[:, 0, :]
                            else:
                                ps = p1ps.tile([128, 512], F32, name="qkvps",
                                               tag="qkvps", bufs=2)
                            terms = ((xh, wh), (xh, wl), (xl, wh))
                            for ti, (xt, wt) in enumerate(terms):
                                for c in range(4):
                                    nc.tensor.matmul(
                                        ps,
                                        xt[:, 2 * c:2 * c + 2,
                                           ts * 128:(ts + 1) * 128],
                                        wt[:, 2 * c:2 * c + 2, :],
                                        start=(ti == 0 and c == 0),
                                        stop=(ti == 2 and c == 3),
                                        perf_mode=mybir.MatmulPerfMode
                                        .DoubleRow)
                            p3 = ps.rearrange("p (h d) -> p h d", h=8)
                            if which == "v":
                                v1t = p1.tile([128, HG], F16, name="v1t",
                                              tag="v1t", bufs=2)
                                nc.sync.dma_start(
                                    out=v1t,
                                    in_=v1_d[tg * 128:(tg + 1) * 128, :])
                                nc.vector.scalar_tensor_tensor(
                                    out=vsb[tg][:, :, 0:64],
                                    in0=p3,
                                    scalar=(1.0 - lam) / 16.0,
                                    in1=v1t.rearrange("p (h d) -> p h d", h=8),
                                    op0=mybir.AluOpType.mult,
                                    op1=mybir.AluOpType.add)
                                nc.vector.tensor_copy(
                                    out=vsb[tg][:, :, 64:65], in_=ones81)
                                continue

                            # ps freed by two quick DVE ops (t1, u); RMS
                            # stats from t1^2+u^2 = ps^2*(c^2+s^2) (scale
                            # invariant, so the 16x fp8 w-scale cancels).
                            c3 = cos_sb[:, tg, :].rearrange(
                                "p (o i) -> p o i", o=1).rearrange(
                                "p h (o d) -> p h o d", o=1).to_broadcast(
                                (128, 8, 2, 32))
                            s3 = sin_sb[:, tg, :].rearrange(
                                "p (o i) -> p o i", o=1).rearrange(
                                "p h (o d) -> p h o d", o=1).to_broadcast(
                                (128, 8, 2, 32))
                            p4 = ps.rearrange("p (h two d) -> p h two d",
                                              two=2, d=32)
                            t1 = p1.tile([128, 8, 2, 32], F16, name="t1",
                                         tag="t1", bufs=2)
                            nc.vector.tensor_mul(out=t1, in0=p4, in1=c3)
                            u = p1.tile([128, 8, 2, 32], F16, name="u",
                                        tag="u", bufs=2)
                            nc.vector.tensor_mul(out=u, in0=p4, in1=s3)
                            sq = p1.tile([128, 512], F32, name="sq",
                                         tag="sq", bufs=2)
                            nc.scalar.square(out=sq, in_=ps)
                            ssum = p1.tile([128, 8], F32, name="ssum",
                                           tag="ssum", bufs=4)
                            nc.vector.tensor_reduce(
                                ssum, sq.rearrange("p (h d) -> p h d", h=8),
                                axis=mybir.AxisListType.X,
                                op=mybir.AluOpType.add)
                            srt = p1.tile([128, 8], F32, name="srt", tag="srt",
                                          bufs=4)
                            nc.scalar.activation(
                                srt, ssum, mybir.ActivationFunctionType.Sqrt,
                                bias=epsc, scale=1.0 / 64.0)
                            rst = p1.tile([128, 8], F32, name="rst", tag="rst",
                                          bufs=4)
                            nc.vector.reciprocal(out=rst, in_=srt)
                            # rot = [t1_0 + u_1 | t1_1 - u_0]  (Pool)
                            rot = p1.tile([128, 8, 2, 32], F16,
                                          name=f"rot{which}",
                                          tag=f"rot{which}", bufs=2)
                            nc.gpsimd.tensor_add(
                                out=rot[:, :, 0, :], in0=t1[:, :, 0, :],
                                in1=u[:, :, 1, :])
                            nc.gpsimd.tensor_sub(
                                out=rot[:, :, 1, :], in0=t1[:, :, 1, :],
                                in1=u[:, :, 0, :])
                            # apply 1/rms (Pool), then DMA-transpose
                            rstb = rst.rearrange(
                                "p (h o) -> p h o", o=1).rearrange(
                                "p h (o d) -> p h o d", o=1).to_broadcast(
                                (128, 8, 2, 32))
                            rot2 = p1.tile([128, 8, 2, 32], F16,
                                           name=f"rr{which}",
                                           tag=f"rr{which}", bufs=2)
                            nc.gpsimd.tensor_mul(out=rot2, in0=rot, in1=rstb)
                            dstT = qT if which == "q" else kT
                            nc.sync.dma_start_transpose(
                                out=dstT[:, :, tg * 128:(tg + 1) * 128],
                                in_=rot2.rearrange("p h two d -> p (h two d)"))

                yT_of = {}
                pending = [None]

                def p2_gen(qc):
                    yT = p2.tile([128, 4, 512], F16, name="yT", tag="yT",
                                 bufs=3)
                    yT_of[qc] = yT
                    for hp in range(4):
                        pair = (2 * hp, 2 * hp + 1)
                        kts = list(range(4 * qc + 4))
                        lag = 3 if len(kts) > 3 else 2
                        pv = {}
                        for h in pair:
                            pv[h] = p1ps.tile([65, 512], F32, name="pv",
                                              tag="pv", bufs=2)
                        pt_live = {}

                        def emit_pv(kt, idx, pv=pv, pair=pair, kts=kts,
                                    qc=qc):
                            m = kt - 4 * qc
                            e0 = 128 * m if m > 0 else 0
                            pt = pt_live.pop(kt)
                            for si, h in enumerate(pair):
                                nc.tensor.matmul(
                                    pv[h][:, e0:512], vsb[kt][:, h, :],
                                    pt[:, si, e0:512],
                                    start=(idx == 0),
                                    stop=(idx == len(kts) - 1))

                        for idx, kt in enumerate(kts):
                            m = kt - 4 * qc
                            a0 = 128 * m if m > 0 else 0
                            st2 = p1ps.tile([128, 2, 512], F32, name="st",
                                            tag="st", bufs=2)
                            for si in range(2):
                                b0 = 64 * si
                                nc.tensor.matmul(
                                    st2[:, si, a0:512],
                                    kT[b0:b0 + 64, hp,
                                       kt * 128:(kt + 1) * 128],
                                    qT[b0:b0 + 64, hp,
                                       qc * 512 + a0:(qc + 1) * 512],
                                    start=True, stop=True)
                            pt = p2.tile([128, 2, 512], F16, name="pt",
                                         tag="pt", bufs=8)
                            nc.scalar.activation(
                                pt[:, :, a0:512], st2[:, :, a0:512],
                                mybir.ActivationFunctionType.Exp,
                                scale=SCALE)
                            if m >= 0:
                                w0 = 128 * m
                                for si in range(2):
                                    nc.vector.tensor_mul(
                                        out=pt[:, si, w0:w0 + 128],
                                        in0=pt[:, si, w0:w0 + 128],
                                        in1=tri01)
                            pt_live[kt] = pt
                            if idx == 3 and pending[0] is not None:
                                pending[0]()
                                pending[0] = None
                            if idx >= lag:
                                emit_pv(kts[idx - lag], idx - lag)
                            yield
                        for j in range(lag, 0, -1):
                            emit_pv(kts[-j], len(kts) - j)
                        if pending[0] is not None:
                            pending[0]()
                            pending[0] = None

                        # reciprocals now (DVE starts while the next pair's
                        # scores stream); broadcast + normalize deferred
                        # into the next pair's loop
                        recs = {}
                        for si, h in enumerate(pair):
                            rec = p2.tile([1, 512], F16, name="rec",
                                          tag="rec", bufs=4)
                            with nc.allow_low_precision(
                                    reason="softmax denom recip fp16"):
                                nc.vector.reciprocal(
                                    out=rec, in_=pv[h][64:65, :])
                            recs[h] = rec

                        def normalize(pv=pv, pair=pair, hp=hp, yT=yT,
                                      recs=recs):
                            bc_ps = p1ps.tile([128, 512], F32, name="bcps",
                                              tag="qkvps", bufs=2)
                            for si, h in enumerate(pair):
                                b0 = 64 * si
                                nc.tensor.matmul(bc_ps[b0:b0 + 64, :],
                                                 ones64, recs[h],
                                                 start=True, stop=True)
                            bc = p2.tile([128, 512], F16, name="bc", tag="bc",
                                         bufs=3)
                            nc.vector.tensor_copy(out=bc, in_=bc_ps)
                            for si, h in enumerate(pair):
                                b0 = 64 * si
                                nc.vector.tensor_mul(
                                    out=yT[b0:b0 + 64, hp, :],
                                    in0=pv[h][0:64, :], in1=bc[b0:b0 + 64, :])

                        pending[0] = normalize

                def proj_gen(qc):
                    yT = yT_of[qc]
                    for tsub in range(4):
                        for jc in range(2):
                            yield
                            pr = p1ps.tile([128, 512], F32, name="pr",
                                           tag="qkvps", bufs=2)
                            for ft in range(4):
                                nc.tensor.matmul(
                                    pr,
                                    yT[:, ft, tsub * 128:(tsub + 1) * 128],
                                    wp_sb[:, ft, jc * 512:(jc + 1) * 512],
                                    start=(ft == 0), stop=(ft == 3))
                            osb = p2.tile([128, 512], F16, name="osb",
                                          tag="osb", bufs=4)
                            nc.vector.tensor_copy(out=osb, in_=pr)
                            r0 = qc * 512 + tsub * 128
                            nc.sync.dma_start(
                                out=out_d[r0:r0 + 128,
                                          jc * 512:(jc + 1) * 512],
                                in_=osb)

                # software pipeline: start q,k of chunk 0; interleave the
                # v units + next chunk's QKV + previous chunk's projection
                # into each attention chunk's kt-step stream.
                g0 = p1_gen(0, xtiles=(x0h, x0l), use_st=True)
                for _ in range(9):      # all q and k units
                    next(g0)
                fill_counts = {0: 4 + 13, 1: 13 + 8, 2: 13, 3: 16}
                for qc in range(NQ):
                    chain = []
                    if qc == 0:
                        chain.append(g0)
                    if qc + 1 < NQ:
                        chain.append(p1_gen(qc + 1))
                    if qc == 1:
                        chain.append(proj_gen(0))
                    if qc == 3:
                        chain.append(proj_gen(1))
                        chain.append(proj_gen(2))
                    if not wp_loaded[0]:
                        nc.sync.dma_start(
                            out=wp_sb,
                            in_=wp_d.rearrange("(c p) j -> p c j", p=128))
                        wp_loaded[0] = True
                    steps = 4 * (4 * qc + 4)
                    nfill = fill_counts[qc]
                    acc = [0.0]
                    rate = nfill / steps

                    def fire():
                        while chain:
                            try:
                                next(chain[0])
                                return
                            except StopIteration:
                                chain.pop(0)

                    if qc == 0:
                        # fill the transpose-latency hole before step 0:
                        # v units + first next-chunk QKV units run on PE
                        # while chunk-0 q/k drain through DVE/Pool/DMA
                        for _ in range(6):
                            fire()
                    i = 0
                    for _ in p2_gen(qc):
                        i += 1
                        acc[0] += rate
                        while acc[0] >= 1.0:
                            acc[0] -= 1.0
                            fire()
                    while chain:
                        fire()
                        if not chain:
                            break
                if pending[0] is not None:
                    pending[0]()
                    pending[0] = None
                for _ in proj_gen(NQ - 1):
                    pass

    _legalize_waits(nc)
    return nc


def _host_tables():
    inv_freq = 1.0 / (10000.0 ** (np.arange(0, D, 2, dtype=np.float32) / D))
    t = np.arange(T, dtype=np.float32)
    freqs = np.outer(t, inv_freq).astype(np.float32)      # (T, 32)
    cos16 = np.cos(freqs).astype(np.float16)
    sin16 = np.sin(freqs).astype(np.float16)
    p = np.arange(128)[:, None]
    f = np.arange(128)[None, :]
    tri = (p <= f).astype(np.float16)                      # (128, 128)
    return cos16, sin16, tri


def _hilo(a):
    hi = a.astype(ml_dtypes.float8_e4m3)
    lo = (a - hi.astype(np.float32)).astype(ml_dtypes.float8_e4m3)
    return hi, lo


_CACHE = {}


def kernel(x, v1, wq, wk, wv, wproj, lamb):
    x = np.asarray(x, dtype=np.float32)
    v1 = np.asarray(v1, dtype=np.float32)
    wq = np.asarray(wq, dtype=np.float32)
    wk = np.asarray(wk, dtype=np.float32)
    wv = np.asarray(wv, dtype=np.float32)
    wproj = np.asarray(wproj, dtype=np.float32)
    lam = float(np.asarray(lamb))

    cosn, sinn, tri = _host_tables()

    key = lam
    if key not in _CACHE:
        _CACHE[key] = _build(lam)
    nc = _CACHE[key]

    in_maps = []
    for core in range(8):
        b, hg = core // 2, core % 2
        sl = slice(hg * HG, (hg + 1) * HG)
        xh, xl = _hilo(np.ascontiguousarray(x[b].T))
        m = {
            "xTh": xh,
            "xTl": xl,
            "v1h": np.ascontiguousarray(
                (lam * v1[b][:, sl]).astype(np.float16)),
            "wpT": np.ascontiguousarray(wproj[:, sl].T.astype(np.float16)),
            "cosn": cosn,
            "sinn": sinn,
            "tri01": tri,
        }
        for wn, w in (("q", wq), ("k", wk), ("v", wv)):
            wh, wl = _hilo(np.ascontiguousarray(w[sl, :].T) * 16.0)
            m[f"w{wn}h"] = wh
            m[f"w{wn}l"] = wl
        in_maps.append(m)

    res = bass_utils.run_bass_kernel_spmd(nc, in_maps, core_ids=list(range(8)))
    y = np.empty((B, T, C), dtype=np.float32)
    for b in range(B):
        y[b] = (res.results[2 * b]["out"].astype(np.float32)
                + res.results[2 * b + 1]["out"].astype(np.float32))
    return (y, v1)


# revision 74
# speedup vs baseline: 1.3559x; 1.0116x over previous
"""Causal self-attention (B=4, T=2048, C=1024, H=16, D=64) on 8 TRN2 cores.

Sharding: core = 2*b + hg  (b = batch 0..3, hg = head-group 0..1 of 8 heads).
Each core computes its batch's QKV projections for its 8 heads, RMSNorm+RoPE,
causal attention, and a partial output projection over its head-group's wproj
rows; the two partials per batch are summed on the host.

v3 pipeline:
  QKV projections run as split-high/low fp8e4m3 DoubleRow matmuls
  (x = xh + xl, w = wh + wl quantized on host; ps = xh@wh + xh@wl + xl@wh,
  ~0.3% rms error, 0.75x the fp16 matmul cost).  RMS scale invariance
  absorbs the w*16 fp8 range scaling for q/k; the v-blend divides by 16.
  RoPE+RMS: t1 = ps*cos, u = ps*sin (DVE, frees the PSUM quickly), RMS
  stats from t1^2+u^2 = ps^2 (DVE fp16), rot = [t1_0+u_1 | t1_1-u_0]
  (Pool), 1/rms applied on Pool, then feature-major transpose via the
  XBAR DMA-transpose engine (zero PE cost).
  Attention per (query chunk qc, head pair hp): S^T per head into one
  [128, 2, 512] PSUM tile, ONE merged exp for both heads (ACT), fp16
  triangle mask on diagonal blocks (DVE 4x), PV accumulates [65, W] per
  head up to 6 steps behind; the 65th ones-column of v computes softmax
  denominators in the same matmul.  Each head-pair's normalization
  (reciprocal -> ones-matmul broadcast -> fp16 yT) is deferred into the
  next pair's loop; output projections are deferred into later chunks'
  attention steps (proj 0 -> chunk 1, proj 1 and 2 -> chunk 3), as are
  the next chunk's QKV units, so PE always has fill work during
  exp-limited stretches.

The ISA has ONE semaphore-wait slot per instruction; Tile emits more.
_legalize_waits() splits extras onto same-engine NoOps post-scheduling.
DmaTransposeAnt cannot encode any wait - all its waits move to NoOps.
"""

import math

import numpy as np
import ml_dtypes

import concourse.bass as bass
import concourse.mybir as mybir
import concourse.tile as tile
from concourse import bass_utils

F32 = mybir.dt.float32
F16 = mybir.dt.float16
F8 = mybir.dt.float8e4

B, T, C, H, D = 4, 2048, 1024, 16, 64
HG = C // 2          # 512 features per head group (8 heads x 64)
NT = T // 128        # 16 t-tiles
NQ = T // 512        # 4 query/t chunks
EPS = 1.1920928955078125e-07
SCALE = 1.0 / math.sqrt(D)  # 0.125

_wsplit_counter = [0]


def _legalize_waits(nc):
    """Split multi-wait instructions into single-wait NoOp chains."""
    n = 0
    for f in nc.m.functions:
        for bb in f.blocks:
            new_list = []
            changed = False
            for inst in bb.instructions:
                si = inst.sync_info
                is_dt = type(inst).__name__ == "InstDmaTransposeAnt"
                keep = 0 if is_dt else 1
                if si is not None and si.on_wait and len(si.on_wait) > keep:
                    waits = list(si.on_wait)
                    for w in (waits if is_dt else waits[:-1]):
                        _wsplit_counter[0] += 1
                        new_list.append(mybir.InstNoOp(
                            name=f"WSPLIT-{_wsplit_counter[0]}",
                            engine=inst.engine, ins=[], outs=[],
                            sync_info=mybir.SyncInfo(on_wait=[w], on_update=[]),
                        ))
                    si.on_wait = [] if is_dt else waits[-1:]
                    changed = True
                    n += 1
                new_list.append(inst)
            if changed:
                bb.instructions = new_list
    return n


def _build(lam: float) -> bass.Bass:
    nc = bass.Bass("TRN2", target_bir_lowering=False, debug=False,
                   num_devices=8)

    xh_d = nc.dram_tensor("xTh", [C, T], F8, kind="ExternalInput").ap()
    xl_d = nc.dram_tensor("xTl", [C, T], F8, kind="ExternalInput").ap()
    v1_d = nc.dram_tensor("v1h", [T, HG], F16, kind="ExternalInput").ap()
    w_d = {}
    for wn in ("q", "k", "v"):
        for piece in ("h", "l"):
            w_d[wn + piece] = nc.dram_tensor(
                f"w{wn}{piece}", [C, HG], F8, kind="ExternalInput").ap()
    wp_d = nc.dram_tensor("wpT", [HG, C], F16, kind="ExternalInput").ap()
    cs_d = nc.dram_tensor("csn", [T, 128], F16, kind="ExternalInput").ap()
    tri_d = nc.dram_tensor("tri01", [128, 128], F16, kind="ExternalInput").ap()
    out_d = nc.dram_tensor("out", [T, C], F16, kind="ExternalOutput").ap()

    with tile.TileContext(nc) as tc:
        with (
            tc.tile_pool(name="const", bufs=1) as const,
            tc.tile_pool(name="pers", bufs=1) as pers,
        ):
            tri01 = const.tile([128, 128], F16)
            nc.gpsimd.dma_start(out=tri01, in_=tri_d)
            ones81 = const.tile([128, 8, 1], F16)
            nc.vector.memset(ones81, 1.0)
            epsc = const.tile([128, 1], F32)
            nc.vector.memset(epsc, EPS)
            ones64 = const.tile([1, 64], F16)
            nc.vector.memset(ones64, 1.0)

            # persistent feature-major q/k and v tiles
            qT = pers.tile([128, 4, T], F16, name="qT", tag="qT")
            kT = pers.tile([128, 4, T], F16, name="kT", tag="kT")
            vsb = [pers.tile([128, 8, 65], F16, name=f"v{t}", tag=f"v{t}")
                   for t in range(NT)]

            with (
                tc.tile_pool(name="p1", bufs=1) as p1,
                tc.tile_pool(name="p1ps", bufs=1, space="PSUM") as p1ps,
                tc.tile_pool(name="p2", bufs=1) as p2,
            ):
                # chunk-0 x DMAs first: the very first matmul needs them
                x0h = p1.tile([128, 8, 512], F8, name="xh", tag="xh", bufs=2)
                x0l = p1.tile([128, 8, 512], F8, name="xl", tag="xl", bufs=2)
                x0hr = xh_d[:, 0:512].rearrange("(c p) t -> p c t", p=128)
                x0lr = xl_d[:, 0:512].rearrange("(c p) t -> p c t", p=128)
                nc.sync.dma_start(out=x0h[:, 0:2, :], in_=x0hr[:, 0:2, :])
                nc.sync.dma_start(out=x0l[:, 0:2, :], in_=x0lr[:, 0:2, :])
                nc.sync.dma_start(out=x0h[:, 2:8, :], in_=x0hr[:, 2:8, :])
                nc.sync.dma_start(out=x0l[:, 2:8, :], in_=x0lr[:, 2:8, :])

                w_sb = {}
                for i, wn in enumerate(("qh", "ql", "kh", "kl", "vh", "vl")):
                    w_sb[wn] = p1.tile([128, 8, HG], F8, name=f"w{wn}")
                    eng = (nc.scalar, nc.gpsimd)[i % 2]
                    wr = w_d[wn[0] + wn[1]].rearrange("(c p) i -> p c i",
                                                      p=128)
                    if wn in ("qh", "ql"):
                        eng.dma_start(out=w_sb[wn][:, 0:2, :],
                                      in_=wr[:, 0:2, :])
                        eng.dma_start(out=w_sb[wn][:, 2:8, :],
                                      in_=wr[:, 2:8, :])
                    else:
                        eng.dma_start(out=w_sb[wn], in_=wr)
                cs_sb = p1.tile([128, NT, 128], F16)
                nc.gpsimd.dma_start(
                    out=cs_sb, in_=cs_d.rearrange("(n p) i -> p n i", p=128))
                wp_sb = p2.tile([128, 4, C], F16)
                wp_loaded = [False]

                def p1_gen(tc4, xtiles=None, use_st=False):
                    if xtiles is not None:
                        xh, xl = xtiles
                    else:
                        xh = p1.tile([128, 8, 512], F8, name="xh", tag="xh",
                                     bufs=2)
                        xl = p1.tile([128, 8, 512], F8, name="xl", tag="xl",
                                     bufs=2)
                        t0 = tc4 * 512
                        nc.sync.dma_start(
                            out=xh, in_=xh_d[:, t0:t0 + 512].rearrange(
                                "(c p) t -> p c t", p=128))
                        nc.sync.dma_start(
                            out=xl, in_=xl_d[:, t0:t0 + 512].rearrange(
                                "(c p) t -> p c t", p=128))
                    for which in ("q", "k", "v"):
                        for ts in range(4):
                            tg = tc4 * 4 + ts
                            yield
                            wh = w_sb[which + "h"]
                            wl = w_sb[which + "l"]
                            if use_st and (ts % 2 == 0):
                                ps = p1ps.tile([128, 2, 512], F32, name="st",
                                               tag="st", bufs=2)[:, 0, :]
                            else:
                                ps = p1ps.tile([128, 512], F32, name="qkvps",
                                               tag="qkvps", bufs=2)
                            terms = ((xh, wh), (xh, wl), (xl, wh))
                            for ti, (xt, wt) in enumerate(terms):
                                for c in range(4):
                                    nc.tensor.matmul(
                                        ps,
                                        xt[:, 2 * c:2 * c + 2,
                                           ts * 128:(ts + 1) * 128],
                                        wt[:, 2 * c:2 * c + 2, :],
                                        start=(ti == 0 and c == 0),
                                        stop=(ti == 2 and c == 3),
                                        perf_mode=mybir.MatmulPerfMode
                                        .DoubleRow)
                            p3 = ps.rearrange("p (h d) -> p h d", h=8)
                            if which == "v":
                                v1t = p1.tile([128, HG], F16, name="v1t",
                                              tag="v1t", bufs=2)
                                nc.sync.dma_start(
                                    out=v1t,
                                    in_=v1_d[tg * 128:(tg + 1) * 128, :])
                                nc.vector.scalar_tensor_tensor(
                                    out=vsb[tg][:, :, 0:64],
                                    in0=p3,
                                    scalar=(1.0 - lam) / 16.0,
                                    in1=v1t.rearrange("p (h d) -> p h d", h=8),
                                    op0=mybir.AluOpType.mult,
                                    op1=mybir.AluOpType.add)
                                nc.vector.tensor_copy(
                                    out=vsb[tg][:, :, 64:65], in_=ones81)
                                continue

                            # ps freed by two quick DVE ops (t1, u); RMS
                            # stats from t1^2+u^2 = ps^2*(c^2+s^2) (scale
                            # invariant, so the 16x fp8 w-scale cancels).
                            cs3 = cs_sb[:, tg, :].rearrange(
                                "p (o cs dd) -> p o cs dd",
                                o=1, cs=2).to_broadcast((128, 8, 2, 64))
                            p5 = ps.rearrange(
                                "p (h dd) -> p h dd", h=8).rearrange(
                                "p h (o dd) -> p h o dd",
                                o=1).to_broadcast((128, 8, 2, 64))
                            tu = p1.tile([128, 8, 2, 64], F16, name="tu",
                                         tag="tu", bufs=2)
                            nc.vector.tensor_mul(out=tu, in0=p5, in1=cs3)
                            t1 = tu[:, :, 0, :].rearrange(
                                "p h (two d) -> p h two d", two=2)
                            u = tu[:, :, 1, :].rearrange(
                                "p h (two d) -> p h two d", two=2)
                            sq = p1.tile([128, 512], F32, name="sq",
                                         tag="sq", bufs=2)
                            nc.scalar.square(out=sq, in_=ps)
                            ssum = p1.tile([128, 8], F32, name="ssum",
                                           tag="ssum", bufs=4)
                            nc.vector.tensor_reduce(
                                ssum, sq.rearrange("p (h d) -> p h d", h=8),
                                axis=mybir.AxisListType.X,
                                op=mybir.AluOpType.add)
                            srt = p1.tile([128, 8], F32, name="srt", tag="srt",
                                          bufs=4)
                            nc.scalar.activation(
                                srt, ssum, mybir.ActivationFunctionType.Sqrt,
                                bias=epsc, scale=1.0 / 64.0)
                            rst = p1.tile([128, 8], F32, name="rst", tag="rst",
                                          bufs=4)
                            nc.vector.reciprocal(out=rst, in_=srt)
                            # rot = [t1_0 + u_1 | t1_1 - u_0]  (Pool)
                            rot = p1.tile([128, 8, 2, 32], F16,
                                          name=f"rot{which}",
                                          tag=f"rot{which}", bufs=2)
                            nc.gpsimd.tensor_add(
                                out=rot[:, :, 0, :], in0=t1[:, :, 0, :],
                                in1=u[:, :, 1, :])
                            nc.gpsimd.tensor_sub(
                                out=rot[:, :, 1, :], in0=t1[:, :, 1, :],
                                in1=u[:, :, 0, :])
                            # apply 1/rms (Pool), then DMA-transpose
                            rstb = rst.rearrange(
                                "p (h o) -> p h o", o=1).rearrange(
                                "p h (o d) -> p h o d", o=1).to_broadcast(
                                (128, 8, 2, 32))
                            rot2 = p1.tile([128, 8, 2, 32], F16,
                                           name=f"rr{which}",
                                           tag=f"rr{which}", bufs=2)
                            nc.gpsimd.tensor_mul(out=rot2, in0=rot, in1=rstb)
                            dstT = qT if which == "q" else kT
                            nc.sync.dma_start_transpose(
                                out=dstT[:, :, tg * 128:(tg + 1) * 128],
                                in_=rot2.rearrange("p h two d -> p (h two d)"))

                yT_of = {}
                pending = [None]

                def p2_gen(qc):
                    yT = p2.tile([128, 4, 512], F16, name="yT", tag="yT",
                                 bufs=3)
                    yT_of[qc] = yT
                    for hp in range(4):
                        pair = (2 * hp, 2 * hp + 1)
                        kts = list(range(4 * qc + 4))
                        lag = 3 if len(kts) > 3 else 2
                        pv = {}
                        for h in pair:
                            pv[h] = p1ps.tile([65, 512], F32, name="pv",
                                              tag="pv", bufs=2)
                        pt_live = {}

                        def emit_pv(kt, idx, pv=pv, pair=pair, kts=kts,
                                    qc=qc):
                            m = kt - 4 * qc
                            e0 = 128 * m if m > 0 else 0
                            pt = pt_live.pop(kt)
                            for si, h in enumerate(pair):
                                nc.tensor.matmul(
                                    pv[h][:, e0:512], vsb[kt][:, h, :],
                                    pt[:, si, e0:512],
                                    start=(idx == 0),
                                    stop=(idx == len(kts) - 1))

                        for idx, kt in enumerate(kts):
                            m = kt - 4 * qc
                            a0 = 128 * m if m > 0 else 0
                            st2 = p1ps.tile([128, 2, 512], F32, name="st",
                                            tag="st", bufs=2)
                            for si in range(2):
                                b0 = 64 * si
                                nc.tensor.matmul(
                                    st2[:, si, a0:512],
                                    kT[b0:b0 + 64, hp,
                                       kt * 128:(kt + 1) * 128],
                                    qT[b0:b0 + 64, hp,
                                       qc * 512 + a0:(qc + 1) * 512],
                                    start=True, stop=True)
                            pt = p2.tile([128, 2, 512], F16, name="pt",
                                         tag="pt", bufs=8)
                            nc.scalar.activation(
                                pt[:, :, a0:512], st2[:, :, a0:512],
                                mybir.ActivationFunctionType.Exp,
                                scale=SCALE)
                            if m >= 0:
                                w0 = 128 * m
                                for si in range(2):
                                    nc.vector.tensor_mul(
                                        out=pt[:, si, w0:w0 + 128],
                                        in0=pt[:, si, w0:w0 + 128],
                                        in1=tri01)
                            pt_live[kt] = pt
                            if idx == 3 and pending[0] is not None:
                                pending[0]()
                                pending[0] = None
                            if idx >= lag:
                                emit_pv(kts[idx - lag], idx - lag)
                            yield
                        for j in range(lag, 0, -1):
                            emit_pv(kts[-j], len(kts) - j)
                        if pending[0] is not None:
                            pending[0]()
                            pending[0] = None

                        # reciprocals now (DVE starts while the next pair's
                        # scores stream); broadcast + normalize deferred
                        # into the next pair's loop
                        recs = {}
                        for si, h in enumerate(pair):
                            rec = p2.tile([1, 512], F16, name="rec",
                                          tag="rec", bufs=4)
                            with nc.allow_low_precision(
                                    reason="softmax denom recip fp16"):
                                nc.vector.reciprocal(
                                    out=rec, in_=pv[h][64:65, :])
                            recs[h] = rec

                        def normalize(pv=pv, pair=pair, hp=hp, yT=yT,
                                      recs=recs):
                            bc_ps = p1ps.tile([128, 512], F32, name="bcps",
                                              tag="qkvps", bufs=2)
                            for si, h in enumerate(pair):
                                b0 = 64 * si
                                nc.tensor.matmul(bc_ps[b0:b0 + 64, :],
                                                 ones64, recs[h],
                                                 start=True, stop=True)
                            bc = p2.tile([128, 512], F16, name="bc", tag="bc",
                                         bufs=3)
                            nc.vector.tensor_copy(out=bc, in_=bc_ps)
                            for si, h in enumerate(pair):
                                b0 = 64 * si
                                nc.vector.tensor_mul(
                                    out=yT[b0:b0 + 64, hp, :],
                                    in0=pv[h][0:64, :], in1=bc[b0:b0 + 64, :])

                        pending[0] = normalize

                def proj_gen(qc):
                    yT = yT_of[qc]
                    for tsub in range(4):
                        for jc in range(2):
                            yield
                            pr = p1ps.tile([128, 512], F32, name="pr",
                                           tag="qkvps", bufs=2)
                            for ft in range(4):
                                nc.tensor.matmul(
                                    pr,
                                    yT[:, ft, tsub * 128:(tsub + 1) * 128],
                                    wp_sb[:, ft, jc * 512:(jc + 1) * 512],
                                    start=(ft == 0), stop=(ft == 3))
                            osb = p2.tile([128, 512], F16, name="osb",
                                          tag="osb", bufs=4)
                            nc.vector.tensor_copy(out=osb, in_=pr)
                            r0 = qc * 512 + tsub * 128
                            nc.sync.dma_start(
                                out=out_d[r0:r0 + 128,
                                          jc * 512:(jc + 1) * 512],
                                in_=osb)

                # software pipeline: start q,k of chunk 0; interleave the
                # v units + next chunk's QKV + previous chunk's projection
                # into each attention chunk's kt-step stream.
                g0 = p1_gen(0, xtiles=(x0h, x0l), use_st=True)
                for _ in range(9):      # all q and k units
                    next(g0)
                fill_counts = {0: 4 + 13, 1: 13 + 8, 2: 13, 3: 16}
                for qc in range(NQ):
                    chain = []
                    if qc == 0:
                        chain.append(g0)
                    if qc + 1 < NQ:
                        chain.append(p1_gen(qc + 1))
                    if qc == 1:
                        chain.append(proj_gen(0))
                    if qc == 3:
                        chain.append(proj_gen(1))
                        chain.append(proj_gen(2))
                    if not wp_loaded[0]:
                        nc.sync.dma_start(
                            out=wp_sb,
                            in_=wp_d.rearrange("(c p) j -> p c j", p=128))
                        wp_loaded[0] = True
                    steps = 4 * (4 * qc + 4)
                    nfill = fill_counts[qc]
                    acc = [0.0]
                    rate = nfill / steps

                    def fire():
                        while chain:
                            try:
                                next(chain[0])
                                return
                            except StopIteration:
                                chain.pop(0)

                    if qc == 0:
                        # fill the transpose-latency hole before step 0:
                        # v units + first next-chunk QKV units run on PE
                        # while chunk-0 q/k drain through DVE/Pool/DMA
                        for _ in range(6):
                            fire()
                    i = 0
                    for _ in p2_gen(qc):
                        i += 1
                        acc[0] += rate
                        while acc[0] >= 1.0:
                            acc[0] -= 1.0
                            fire()
                    while chain:
                        fire()
                        if not chain:
                            break
                if pending[0] is not None:
                    pending[0]()
                    pending[0] = None
                for _ in proj_gen(NQ - 1):
                    pass

    _legalize_waits(nc)
    return nc


def _host_tables():
    inv_freq = 1.0 / (10000.0 ** (np.arange(0, D, 2, dtype=np.float32) / D))
    t = np.arange(T, dtype=np.float32)
    freqs = np.outer(t, inv_freq).astype(np.float32)      # (T, 32)
    c = np.cos(freqs)
    sn = np.sin(freqs)
    cs16 = np.concatenate([c, c, sn, sn], axis=1).astype(np.float16)
    p = np.arange(128)[:, None]
    f = np.arange(128)[None, :]
    tri = (p <= f).astype(np.float16)                      # (128, 128)
    return cs16, tri


def _hilo(a):
    hi = a.astype(ml_dtypes.float8_e4m3)
    lo = (a - hi.astype(np.float32)).astype(ml_dtypes.float8_e4m3)
    return hi, lo


_CACHE = {}


def kernel(x, v1, wq, wk, wv, wproj, lamb):
    x = np.asarray(x, dtype=np.float32)
    v1 = np.asarray(v1, dtype=np.float32)
    wq = np.asarray(wq, dtype=np.float32)
    wk = np.asarray(wk, dtype=np.float32)
    wv = np.asarray(wv, dtype=np.float32)
    wproj = np.asarray(wproj, dtype=np.float32)
    lam = float(np.asarray(lamb))

    csn, tri = _host_tables()

    key = lam
    if key not in _CACHE:
        _CACHE[key] = _build(lam)
    nc = _CACHE[key]

    in_maps = []
    for core in range(8):
        b, hg = core // 2, core % 2
        sl = slice(hg * HG, (hg + 1) * HG)
        xh, xl = _hilo(np.ascontiguousarray(x[b].T))
        m = {
            "xTh": xh,
            "xTl": xl,
            "v1h": np.ascontiguousarray(
                (lam * v1[b][:, sl]).astype(np.float16)),
            "wpT": np.ascontiguousarray(wproj[:, sl].T.astype(np.float16)),
            "csn": csn,
            "tri01": tri,
        }
        for wn, w in (("q", wq), ("k", wk), ("v", wv)):
            wh, wl = _hilo(np.ascontiguousarray(w[sl, :].T) * 16.0)
            m[f"w{wn}h"] = wh
            m[f"w{wn}l"] = wl
        in_maps.append(m)

    res = bass_utils.run_bass_kernel_spmd(nc, in_maps, core_ids=list(range(8)))
    y = np.empty((B, T, C), dtype=np.float32)
    for b in range(B):
        y[b] = (res.results[2 * b]["out"].astype(np.float32)
                + res.results[2 * b + 1]["out"].astype(np.float32))
    return (y, v1)
